# revision 30
# baseline (speedup 1.0000x reference)
"""Trainium2 Bass kernel for AdaptiveHierarchicalAttention (8 NeuronCores).

Reference computation (per level l in 0..3):
    x_l = query[:, ::2^l, :]                         # [1, S_l, E], S_l = S >> l
    outs[l] = MHA_l(x_l)                             # 16-head self-attention
Bottom-up: current = outs[3]; for l in (2,1,0):
    current = upsample_linear(current, S_l) @ up_w[l].T + up_b[l] + outs[l]

Sharding (8 cores): tensor-parallel over heads (2 heads/core) for every
level's attention; the up-propagation chain is folded on the host into one
E x E matrix D_l per level (plus a bias vector beta), so the device epilogue
is Z_l = A_l @ D_l at each level's resolution followed by chained 2x
linear-interp upsampling and adds.

Schedule (the critical resources are the PE (~108us of bf16 matmul) and the
Activation engine (~88us of softmax exp); collectives cost 15us fixed each in
the cost model so their issue order decides the tail):
  - Level order: 3, 2 first (their fused exchange is in flight by ~30us and
    hides under level-0 compute), then level 0 pass A (first 128-token half
    of every dest core's block, q-strided), then level 1, then level 0 pass
    B.  Exchanges fire immediately after their producer: a2a32, a2a0a, a2a1,
    a2a0b.  Only the last one (one half of level 0) is exposed, followed by a
    ~6us epilogue tail.
  - V is produced directly token-major (x-chunk as lhsT against the V weight
    block), eliminating the PE transposes and DVE repacks of the baseline; V
    bias folds into the host-side beta (softmax weights sum to 1, so a V bias
    shifts A by a constant that the linear epilogue maps to a constant).
  - QKV projection and epilogue Z_l matmul work is emitted as queues of
    closures drained into the PE bubbles of attention blocks (where PE waits
    on ScalarE exps), with per-call budgets so the Activation engine never
    starves.
  - Attention math: scoresT[k,q] = K^T Q per head via feature-major Q/K
    straight from the projection; exp on ScalarE without max subtraction
    (scores are O(1)); AV token-major with an appended ones column so the
    softmax denominator falls out of the same matmuls; per-partition
    reciprocal broadcast on DVE; PE transpose back to feature-major.

kernel(**inputs) takes the FULL unsharded inputs and returns the FULL output.
"""

import sys

import numpy as np

sys.path.insert(0, "/opt/trn_rl_repo")

import ml_dtypes  # noqa: E402

import concourse.mybir as mybir  # noqa: E402
import concourse.tile as tile  # noqa: E402
from concourse import bacc  # noqa: E402
from concourse.masks import make_identity  # noqa: E402

F32 = mybir.dt.float32
BF16 = mybir.dt.bfloat16
BF16_NP = ml_dtypes.bfloat16

NCORES = 8
LEVELS = 4
P = 128


def _cfg(S=2048, E=1024, H=16):
    c = {}
    c["S"], c["E"], c["H"] = S, E, H
    c["HD"] = E // H                    # head dim
    c["HPC"] = H // NCORES              # heads per core
    c["F"] = c["HPC"] * c["HD"]         # feature rows per core
    assert c["F"] == 128, "per-core feature slice must be 128"
    c["ECH"] = E // P                   # contraction chunks
    c["SL"] = [S >> l for l in range(LEVELS)]
    c["LOFF"] = np.cumsum([0] + c["SL"]).tolist()   # level offsets in token concat
    c["T"] = sum(c["SL"])               # total tokens across levels
    c["CH"] = [sl // P for sl in c["SL"]]
    c["CHOFF"] = np.cumsum([0] + c["CH"]).tolist()
    c["CHT"] = sum(c["CH"])
    c["BLK"] = [sl // NCORES for sl in c["SL"]]     # per-core token block
    # epilogue windows (token ranges incl. halos): level 0 has no halo.
    c["WIN"] = [c["BLK"][0], c["BLK"][1] + 2, c["BLK"][2] + 4, c["BLK"][3] + 4]
    # upsample phase per step l+1 -> l  (True = "even" pattern A)
    c["PHASE_A"] = [True, False, True]  # index by l of target level 0,1,2
    c["PAD"] = 2
    c["QB0"] = min(512, c["SL"][0])     # level-0 q-block width
    return c


# ---------------------------------------------------------------------------
# builder
# ---------------------------------------------------------------------------

def build(cfg, kgroup=8):
    S, E = cfg["S"], cfg["E"]
    HD, F, ECH = cfg["HD"], cfg["F"], cfg["ECH"]
    SL, LOFF, T = cfg["SL"], cfg["LOFF"], cfg["T"]
    CH, CHOFF, CHT = cfg["CH"], cfg["CHOFF"], cfg["CHT"]
    BLK, WIN, PAD = cfg["BLK"], cfg["WIN"], cfg["PAD"]
    QB0 = cfg["QB0"]
    FT = ECH  # number of 128-wide feature tiles of E
    VW = 2 * HD + 4  # V-token chunk width: [V_A | 1 | pad | V_B | 1 | pad]

    nc = bacc.Bacc(
        "TRN2",
        target_bir_lowering=False,
        debug=False,
        enable_asserts=False,
        num_devices=NCORES,
    )

    # --- I/O ---------------------------------------------------------------
    qT = nc.dram_tensor("qT", [E, S], BF16, kind="ExternalInput")
    win_p = nc.dram_tensor("win", [LEVELS, P, 3, ECH, F], BF16, kind="ExternalInput")
    bin_p = nc.dram_tensor("bin", [P, LEVELS, 3], F32, kind="ExternalInput")
    # folded epilogue weights D_l (E x E each) packed for lhsT use, + beta
    wd_p = nc.dram_tensor("wd", [LEVELS, P, ECH, FT, P], BF16, kind="ExternalInput")
    beta_p = nc.dram_tensor("beta", [P, FT], F32, kind="ExternalInput")
    out_p = nc.dram_tensor("out", [E, BLK[0]], F32, kind="ExternalOutput")

    # --- internal DRAM (collective bounce) ---------------------------------
    CW = [SL[3] + 2 * PAD, SL[2] + 2 * PAD, SL[1] + 2 * PAD]
    CO = {3: 0, 2: CW[0], 1: CW[0] + CW[1]}      # concat offset per level
    CTOT = sum(CW)
    HALO = {1: 1, 2: 2, 3: 2}
    W32 = WIN[3] + WIN[2]
    agin32 = nc.dram_tensor("agin32", [NCORES, P, W32], BF16)
    g32 = nc.dram_tensor("g32", [NCORES, P, W32], BF16)
    agin1 = nc.dram_tensor("agin1", [NCORES, P, WIN[1]], BF16)
    g1 = nc.dram_tensor("g1", [NCORES, P, WIN[1]], BF16)
    HB = BLK[0] // 2
    agin0a = nc.dram_tensor("agin0a", [NCORES, P, HB], BF16)
    g0a = nc.dram_tensor("g0a", [NCORES, P, HB], BF16)
    agin0b = nc.dram_tensor("agin0b", [NCORES, P, HB], BF16)
    g0b = nc.dram_tensor("g0b", [NCORES, P, HB], BF16)
    rg = [list(range(NCORES))]

    with tile.TileContext(nc) as tc:
        from contextlib import ExitStack

        with ExitStack() as ctx:
            pool = lambda name, bufs, **kw: ctx.enter_context(
                tc.tile_pool(name=name, bufs=bufs, **kw)
            )
            const = pool("const", 1)
            d_pool = pool("dw", 1)
            g_pool = pool("gpool", 1)
            qk_pool = pool("qk", 1)
            at_pool = pool("at", 16)
            nrm_pool = pool("nrm", 4)
            a0_pool = pool("a0", 3)
            # QKV weights die after the last projection (end of pass A);
            # their pool closes there and the epilogue acc pool reuses the
            # space.
            wl_stack = ctx.enter_context(ExitStack())
            wl_pool = wl_stack.enter_context(tc.tile_pool(name="wl", bufs=1))
            qkv_ps = pool("qkv_ps", 2, space="PSUM")
            sc_ps = pool("sc_ps", 2, space="PSUM")
            av_ps = pool("av_ps", 1, space="PSUM")

            # --- constants / persistent buffers ---------------------------
            ident = const.tile([P, P], BF16, tag="ident")
            make_identity(nc, ident[:])
            # f32 identity for the f32 norm transposes (dtypes must match)
            ident32 = const.tile([P, P], F32, tag="ident32")
            nc.vector.tensor_copy(out=ident32[:], in_=ident[:])

            # DMA engine split (transfer time occupies the issuing engine's
            # queue in the cost model): xT is spread over the SP, Pool and
            # Activation queues so the full input is resident by ~7us; Pool
            # then takes the folded-weight prefetch, paused around its
            # collectives so they fire at data-ready.
            wl_sb = wl_pool.tile([P, LEVELS, 3, ECH, F], BF16, tag="wl_sb")
            win_r = win_p.ap().rearrange("l p three c f -> p l three c f")
            xT = qk_pool.tile([P, ECH, S], BF16, tag="xT")
            qT_r = qT.ap().rearrange("(c p) t -> p c t", p=P)
            b_sb = const.tile([P, LEVELS, 3], F32, tag="b_sb")
            # SP: bias, level-3 Q/K weights, xT 0-2, then remaining weights
            nc.sync.dma_start(b_sb[:], bin_p[:])
            nc.sync.dma_start(wl_sb[:, 3, 0], win_r[:, 3, 0])
            nc.sync.dma_start(wl_sb[:, 3, 1], win_r[:, 3, 1])
            for c in (0, 1, 2):
                nc.sync.dma_start(xT[:, c, :], qT_r[:, c, :])
            # Activation: xT 6-7 + level-3 V weights, then the exp-table warm
            for c in (6, 7):
                nc.scalar.dma_start(xT[:, c, :], qT_r[:, c, :])
            nc.scalar.dma_start(wl_sb[:, 3, 2], win_r[:, 3, 2])
            warm = const.tile([P, 2], BF16, tag="warm")
            nc.scalar.activation(
                warm[:], ident[:, 0:2], mybir.ActivationFunctionType.Exp
            )
            # Pool: xT 3-5, level-2 weights, then folded weights for 3+2
            for c in (3, 4, 5):
                nc.gpsimd.dma_start(xT[:, c, :], qT_r[:, c, :])
            nc.gpsimd.dma_start(wl_sb[:, 2], win_r[:, 2])
            for l in (1, 0):
                nc.sync.dma_start(wl_sb[:, l], win_r[:, l])
            beta_sb = const.tile([P, FT], F32, tag="beta_sb")
            nc.sync.dma_start(beta_sb[:], beta_p[:])

            wd_sb = d_pool.tile([P, LEVELS, ECH, FT, P], BF16, tag="wd_sb")
            wd_r = wd_p.ap().rearrange("l p c ft f -> p l c ft f")
            for l in (3, 2):
                for c in range(ECH):
                    nc.gpsimd.dma_start(wd_sb[:, l, c], wd_r[:, l, c])

            Q = qk_pool.tile([P, T], BF16, tag="Q")
            K = qk_pool.tile([P, T], BF16, tag="K")
            # level-0 scores run as fp8 DoubleRow: Q8 holds the fp8 query
            # twice (both DR k-tiles), K8 holds {hi, lo = K - hi} so K keeps
            # near-bf16 precision; only Q's fp8 rounding enters the scores.
            F8 = mybir.dt.float8e4
            Q8 = qk_pool.tile([P, 2, SL[0]], F8, tag="Q8")
            K8 = qk_pool.tile([P, 2, SL[0]], F8, tag="K8")
            Vt = qk_pool.tile([P, CHT, VW], BF16, tag="Vt")
            nc.vector.memset(Vt[:, :, HD : HD + 1], 1.0)
            nc.vector.memset(Vt[:, :, 2 * HD + 2 : 2 * HD + 3], 1.0)

            # ---------------- per-level QKV -------------------------------
            # Emitted as queues of closures, drained into PE bubbles of
            # attention blocks.  Q/K are feature-major; V goes directly
            # token-major into Vt (x-chunk as lhsT, V-weight block as rhs),
            # so no V transposes are needed.  V bias is folded into beta on
            # the host.
            def qkv_chunks(l):
                stride = 1 << l
                sl = SL[l]
                nt = min(512, sl)

                def proj(part, n0, dst):
                    def emit():
                        ps = qkv_ps.tile([F, 512], F32, tag="qkv", name="qkvps")
                        for c in range(ECH):
                            rhs = xT[:, c, n0 * stride : (n0 + nt) * stride : stride]
                            nc.tensor.matmul(
                                ps[:, 0:nt],
                                lhsT=wl_sb[:, l, part, c, :],
                                rhs=rhs,
                                start=(c == 0),
                                stop=(c == ECH - 1),
                            )
                        bias = b_sb[:, l, part : part + 1].to_broadcast((F, nt))
                        if l == 0 and part == 0:
                            # fp8 query, duplicated into both DR k-tiles
                            for t8 in (0, 1):
                                nc.vector.tensor_tensor(
                                    Q8[:, t8, n0 : n0 + nt], ps[:, 0:nt], bias,
                                    mybir.AluOpType.add,
                                )
                            return
                        o = dst[:, LOFF[l] + n0 : LOFF[l] + n0 + nt]
                        nc.vector.tensor_tensor(
                            o, ps[:, 0:nt], bias, mybir.AluOpType.add
                        )
                        if l == 0 and part == 1:
                            # fp8 key pair: hi = fp8(K), lo = fp8(K - hi)
                            nc.vector.tensor_copy(
                                out=K8[:, 0, n0 : n0 + nt], in_=o
                            )
                            nc.vector.tensor_tensor(
                                K8[:, 1, n0 : n0 + nt], o, K8[:, 0, n0 : n0 + nt],
                                mybir.AluOpType.subtract,
                            )

                    return emit

                def vproj(j):
                    def emit():
                        ps = qkv_ps.tile([F, 512], F32, tag="qkv", name="qkvps")
                        out = ps[:, 0:P]          # [128 tok, 128 dim]
                        for c in range(ECH):
                            lhsT = xT[:, c, j * P * stride : (j + 1) * P * stride : stride]
                            nc.tensor.matmul(
                                out,
                                lhsT=lhsT,
                                rhs=wl_sb[:, l, 2, c, :],
                                start=(c == 0),
                                stop=(c == ECH - 1),
                            )
                        src = ps[:, 0:HD]
                        src.ap.insert(1, [HD, 2])
                        ch = CHOFF[l] + j
                        dstv = Vt[:, ch, 0:HD]
                        dstv.ap.insert(1, [HD + 2, 2])
                        nc.vector.tensor_copy(out=dstv, in_=src)

                    return emit

                work = []
                for part, dst in ((0, Q), (1, K)):
                    for n0 in range(0, sl, nt):
                        work.append(proj(part, n0, dst))
                for j in range(CH[l]):
                    work.append(vproj(j))
                return work

            def mk_filler(work, schedule):
                """Filler draining `work` with a per-call budget schedule."""
                state = {"i": 0}

                def filler():
                    k = (
                        schedule[state["i"]]
                        if state["i"] < len(schedule)
                        else (schedule[-1] if schedule else 0)
                    )
                    state["i"] += 1
                    for _ in range(min(k, len(work))):
                        work.pop(0)()

                return filler

            def drain(work):
                while work:
                    work.pop(0)()

            def attn_block(l, qb0, qbw, a_dst, a_off, filler=None, qstride=None):
                """Attention for q-block [qb0, qb0+qbw) of level l -> a_dst[:, a_off:].

                With qstride, the q-set is qbw//128 chunks of 128 tokens at
                stride qstride starting at qb0 (strided level-0 half-blocks).

                Emission interleaves score/exp units with the AV matmuls of
                the unit two steps back (matching the 2-buffer sc_ps
                rotation), so PE trails the exp stream by ~2us with no
                stall cascade.  The filler is called once before the first
                AV (for work that must precede AVs, e.g. V projections) and
                once after each AV group.
                """
                nch = CH[l]
                nqc = qbw // P
                fp8 = l == 0  # scores via fp8 DoubleRow (Q8 x (K_hi+K_lo))

                def qrhs(b, qc=None):
                    if qstride is None:
                        return Q[b : b + HD, LOFF[l] + qb0 : LOFF[l] + qb0 + qbw]
                    if qc is None:
                        r = Q[b : b + HD, LOFF[l] + qb0 : LOFF[l] + qb0 + P]
                        r.ap.insert(1, [qstride, nqc])
                        return r
                    # fp8 DoubleRow path: one contiguous 128-token q-chunk
                    q0 = qb0 + qc * qstride
                    return Q8[b : b + HD, :, q0 : q0 + P]

                avb = av_ps.tile([P, 8, P], F32, tag="avb")
                ats = {}
                seq = []
                for i0 in range(0, nch, 2):
                    for h in (0, 1):
                        seq.append((list(range(i0, min(i0 + 2, nch))), h))

                def emit_sc(pair, h):
                    b = h * HD
                    sp = sc_ps.tile([P, 2 * qbw], F32, tag="sc")
                    for j, kc in enumerate(pair):
                        if fp8:
                            for qc in range(nqc):
                                nc.tensor.matmul(
                                    sp[:, j * qbw + qc * P : j * qbw + (qc + 1) * P],
                                    lhsT=K8[b : b + HD, :, kc * P : (kc + 1) * P],
                                    rhs=qrhs(b, qc),
                                    start=True,
                                    stop=True,
                                    perf_mode=mybir.MatmulPerfMode.DoubleRow,
                                )
                        else:
                            nc.tensor.matmul(
                                sp[:, j * qbw : (j + 1) * qbw],
                                lhsT=K[b : b + HD, LOFF[l] + kc * P : LOFF[l] + (kc + 1) * P],
                                rhs=qrhs(b),
                                start=True,
                                stop=True,
                            )
                    at = at_pool.tile([P, 2 * qbw], BF16, tag="at")
                    nc.scalar.activation(
                        at[:, 0 : len(pair) * qbw],
                        sp[:, 0 : len(pair) * qbw],
                        mybir.ActivationFunctionType.Exp,
                    )
                    for j, kc in enumerate(pair):
                        ats[(kc, h)] = at[:, j * qbw : (j + 1) * qbw]

                def emit_av(pair, h):
                    for kc in pair:
                        for qc in range(nqc):
                            slot = qc * 2 + h
                            c0 = 0 if h == 0 else HD + 2
                            # one start per 2KB PSUM bank: the first write
                            # marks the whole bank pending-zero, the other
                            # slots' first writes self-zero.  stop on the
                            # chronologically last write to the bank.
                            nc.tensor.matmul(
                                avb[:, slot, 0 : HD + 1],
                                lhsT=ats[(kc, h)][:, qc * P : (qc + 1) * P],
                                rhs=Vt[:, CHOFF[l] + kc, c0 : c0 + HD + 1],
                                start=(kc == 0 and h == 0 and qc % 2 == 0),
                                stop=(
                                    kc == nch - 1
                                    and h == 1
                                    and (qc % 2 == 1 or qc == nqc - 1)
                                ),
                            )

                for i, (pair, h) in enumerate(seq):
                    emit_sc(pair, h)
                    if i == 1 and filler is not None:
                        filler()
                    if i >= 2:
                        emit_av(*seq[i - 2])
                        if filler is not None:
                            filler()
                for i in range(max(0, len(seq) - 2), len(seq)):
                    emit_av(*seq[i])
                    if filler is not None:
                        filler()

                # normalize (per-partition reciprocal of the denominator
                # column), pack both heads side by side, transpose back to
                # feature-major via the PE, copy into the destination buffer
                for qc in range(nqc):
                    sb = nrm_pool.tile([P, P], F32, tag="sb")
                    for h in (0, 1):
                        slot = qc * 2 + h
                        r = nrm_pool.tile([P, 1], F32, tag=f"r{h}", name="rcp")
                        nc.vector.reciprocal(
                            r[:, 0:1], avb[:, slot, HD : HD + 1]
                        )
                        nc.vector.tensor_mul(
                            out=sb[:, h * HD : (h + 1) * HD],
                            in0=avb[:, slot, 0:HD],
                            in1=r[:, 0:1].to_broadcast((P, HD)),
                        )
                    # f32 transpose through the 2-buffer score PSUM pool so
                    # consecutive qc transposes pipeline with the copies
                    tp = sc_ps.tile([P, 2 * qbw], F32, tag="sc", name="trp")
                    nc.tensor.transpose(tp[:, 0:P], sb[:], ident32[:])
                    nc.vector.tensor_copy(
                        out=a_dst[:, a_off + qc * P : a_off + (qc + 1) * P],
                        in_=tp[:, 0:P],
                    )

            A123 = qk_pool.tile([P, CTOT], BF16, tag="A123")

            def pad_edges(l):
                sl = SL[l]
                co = CO[l]
                nc.vector.tensor_copy(
                    out=A123[:, co : co + PAD],
                    in_=A123[:, co + PAD : co + PAD + 1].to_broadcast((P, PAD)),
                )
                nc.vector.tensor_copy(
                    out=A123[:, co + PAD + sl : co + 2 * PAD + sl],
                    in_=A123[:, co + PAD + sl - 1 : co + PAD + sl].to_broadcast((P, PAD)),
                )

            def attn_level_whole(l, filler):
                sl = SL[l]
                co = CO[l]
                qbw = min(512, sl)
                for qb0 in range(0, sl, qbw):
                    attn_block(l, qb0, qbw, A123, co + PAD + qb0, filler=filler)
                pad_edges(l)

            def bounce_windows(l, dst_dram, woff):
                """One DMA materializing all 8 overlapping dest windows."""
                s0 = CO[l] + PAD - HALO[l]
                src = A123[:, s0 : s0 + WIN[l]]
                src.ap.insert(1, [BLK[l], NCORES])
                dst = dst_dram.ap().rearrange("d p w -> p d w")
                nc.sync.dma_start(dst[:, :, woff : woff + WIN[l]], src)

            def a2a(ins_t, outs_t):
                nc.gpsimd.collective_compute(
                    "AllToAll",
                    mybir.AluOpType.bypass,
                    replica_groups=rg,
                    ins=[ins_t[:]],
                    outs=[outs_t[:]],
                )

            def attn_level0_pass(filler, half, ag, gout):
                """One strided half-pass of level 0: the q-set is the
                `half`-th 128-token half of every dest core's 256-block."""
                for b in range(2):
                    A0 = a0_pool.tile([P, QB0], BF16, tag="A0")
                    attn_block(
                        0,
                        b * 4 * BLK[0] + half * HB,
                        QB0,
                        A0,
                        0,
                        filler=filler,
                        qstride=BLK[0],
                    )
                    # 4 dests x 128 cols per block
                    nc.sync.dma_start(
                        ag.ap()[b * 4 : (b + 1) * 4].rearrange("d p w -> p d w"),
                        A0[:].rearrange("p (d w) -> p d w", d=4),
                    )
                a2a(ag, gout)

            # ---------------- epilogue work units -------------------------
            # Z_l = (gathered A_l window) @ D_l at level-l resolution, per
            # 128-wide output feature tile; emitted as PE filler closures.
            def z_units(gtile, goff, l, w, fin, tail=False):
                """One closure per ft: matmul into psum, then fin(ft, ps).

                tail=True uses the (then idle) 2-buffer score PSUM pool so
                consecutive units pipeline instead of ping-ponging on the
                single qkv bank.
                """
                units = []
                for ft in range(FT):
                    def emit(ft=ft):
                        if tail:
                            ps = sc_ps.tile([P, 1024], F32, tag="sc", name="zps")
                        else:
                            ps = qkv_ps.tile([F, 512], F32, tag="qkv", name="qkvps")
                        for c in range(ECH):
                            nc.tensor.matmul(
                                ps[:, 0:w],
                                lhsT=wd_sb[:, l, c, ft],
                                rhs=gtile[:, c, goff : goff + w],
                                start=(c == 0),
                                stop=(c == ECH - 1),
                            )
                        fin(ft, ps[:, 0:w])
                    units.append(emit)
                return units

            def upsample(cur, ws, w, phase_a, tag):
                """2x linear-interp upsample [P, FT, ws] -> [P, FT, w] (DVE)."""
                p25 = acc_pool.tile([P, FT, ws], BF16, tag=f"p25{tag}")
                p75 = acc_pool.tile([P, FT, ws], BF16, tag=f"p75{tag}")
                nc.vector.tensor_scalar_mul(p25[:], cur[:], 0.25)
                nc.vector.tensor_scalar_mul(p75[:], cur[:], 0.75)
                up = acc_pool.tile([P, FT, w], BF16, tag=f"up{tag}")
                hw = (w + 1) // 2
                hw2 = w // 2
                if phase_a:
                    nc.vector.tensor_add(
                        up[:, :, 0::2], p25[:, :, 0:hw], p75[:, :, 1 : hw + 1]
                    )
                    nc.vector.tensor_add(
                        up[:, :, 1::2], p75[:, :, 1 : hw2 + 1], p25[:, :, 2 : hw2 + 2]
                    )
                else:
                    nc.vector.tensor_add(
                        up[:, :, 0::2], p75[:, :, 1 : hw + 1], p25[:, :, 2 : hw + 2]
                    )
                    nc.vector.tensor_add(
                        up[:, :, 1::2], p25[:, :, 1 : hw2 + 1], p75[:, :, 2 : hw2 + 2]
                    )
                return up

            # ---------------- schedule ------------------------------------
            # Attention order: 3, 2, 1, 0A, 0B.  Exchange order: a2a32,
            # a2a1, a2a0a, a2a0b -- each fires at data-ready into a free
            # slot on the Pool queue; only a2a0b is exposed at the tail.
            w3 = qkv_chunks(3)
            w2 = qkv_chunks(2)
            w1 = qkv_chunks(1)
            w0 = qkv_chunks(0)
            w1qk, w1v = w1[:4], w1[4:]
            w0qk, w0v = w0[:8], w0[8:]

            drain(w3)
            # level 3 attention; fill with level-2 QKV (6 units)
            attn_level_whole(3, mk_filler(w2, [2, 2, 2]))
            drain(w2)
            bounce_windows(3, agin32, 0)
            # level 2 attention; fill with level-1 Q/K
            attn_level_whole(2, mk_filler(w1qk, [0, 1, 1, 1, 1]))
            drain(w1qk)
            bounce_windows(2, agin32, WIN[3])
            a2a(agin32, g32)
            Gs32 = g_pool.tile([P, ECH, W32], BF16, tag="gs32")
            nc.sync.dma_start(Gs32[:], g32.ap().rearrange("b p t -> p b t"))

            # level 1 attention: V of L1 in the pre-AV slot (needed before
            # its AVs), then level-0 Q/K and level-0 V one unit per AV slot.
            wB = w1v + w0qk + w0v
            fB = mk_filler(wB, [8] + [1] * 30)
            attn_block(1, 0, 512, A123, CO[1] + PAD, filler=fB)
            attn_block(1, 512, 512, A123, CO[1] + PAD + 512, filler=fB)
            drain(wB)
            pad_edges(1)
            bounce_windows(1, agin1, 0)
            a2a(agin1, g1)
            Gs1 = g_pool.tile([P, ECH, WIN[1]], BF16, tag="gs1")
            nc.sync.dma_start(Gs1[:], g1.ap().rearrange("b p t -> p b t"))
            # folded weights for levels 1, 0 prefetch behind a2a1 on Pool
            for l in (1, 0):
                for c in range(ECH):
                    nc.gpsimd.dma_start(wd_sb[:, l, c], wd_r[:, l, c])

            # level 0 pass A; Z3 epilogue matmuls spread over the second
            # block's AV slots (Gs32 has landed by then).
            zt3 = g_pool.tile([P, FT, WIN[3]], BF16, tag="zt3")

            def fin3(ft, ps):
                nc.vector.tensor_copy(out=zt3[:, ft, :], in_=ps)

            z3u = z_units(Gs32, 0, 3, WIN[3], fin3)
            attn_level0_pass(mk_filler(z3u, [0] * 18 + [1] * 16), 0, agin0a, g0a)
            drain(z3u)
            wl_stack.close()
            acc_pool = pool("acc", 1)
            Gs0a = g_pool.tile([P, ECH, HB], BF16, tag="gs0a")
            nc.sync.dma_start(Gs0a[:], g0a.ap().rearrange("b p t -> p b t"))
            up_a = upsample(zt3, WIN[3], WIN[2], cfg["PHASE_A"][2], "a")

            # level 0 pass B with the Z2 then (up_b) then Z1 epilogue as
            # fillers; Gs1 lands early in this window.
            acc2 = acc_pool.tile([P, FT, WIN[2]], BF16, tag="acc2")
            acc1 = acc_pool.tile([P, FT, WIN[1]], BF16, tag="acc1")

            def fin2(ft, ps):
                nc.vector.tensor_tensor(
                    acc2[:, ft, :], ps, up_a[:, ft, :], mybir.AluOpType.add
                )

            def fin1(ft, ps):
                nc.vector.tensor_tensor(
                    acc1[:, ft, :], ps, up_b[:, ft, :], mybir.AluOpType.add
                )

            z2u = z_units(Gs32, WIN[3], 2, WIN[2], fin2)
            up_b = None

            def emit_up_b():
                nonlocal up_b
                up_b = upsample(acc2, WIN[2], WIN[1], cfg["PHASE_A"][1], "b")

            z1u = z_units(Gs1, 0, 1, WIN[1], fin1)
            wC = z2u + [emit_up_b] + z1u
            attn_level0_pass(mk_filler(wC, [0] + [1] * 40), 1, agin0b, g0b)
            drain(wC)

            # half-A epilogue + chain to upc while the last exchange flies
            o_a = acc_pool.tile([P, FT, HB], F32, tag="o_a")

            def fin0a(ft, ps):
                nc.vector.tensor_tensor(
                    o_a[:, ft, :],
                    ps,
                    beta_sb[:, ft : ft + 1].to_broadcast((P, HB)),
                    mybir.AluOpType.add,
                )

            drain(z_units(Gs0a, 0, 0, HB, fin0a, tail=True))
            upc = upsample(acc1, WIN[1], WIN[0], cfg["PHASE_A"][0], "c")

            # output half A: final add + streamed per-ft DMA.  The output
            # DMAs go on the Activation queue (its exps are done by now),
            # keeping SP free and the tail off the critical path.
            out_r = out_p.ap().rearrange("(c p) t -> p c t", p=P)
            for ft in range(FT):
                oa = acc_pool.tile([P, HB], F32, tag=f"oA{ft}", name="o_t")
                nc.vector.tensor_tensor(
                    oa[:], o_a[:, ft, :], upc[:, ft, 0:HB], mybir.AluOpType.add
                )
                nc.scalar.dma_start(out_r[:, ft, 0:HB], oa[:])

            # tail: half B (needs the last exchange); load split over two
            # DMA queues so it lands in ~0.8us.
            gs0b = g_pool.tile([P, ECH, HB], BF16, tag="gs0b")
            g0b_r = g0b.ap().rearrange("b p t -> p b t")
            nc.sync.dma_start(gs0b[:, 0:4], g0b_r[:, 0:4])
            nc.scalar.dma_start(gs0b[:, 4:8], g0b_r[:, 4:8])

            def fin0b(ft, ps):
                ob = acc_pool.tile([P, HB], F32, tag=f"oB{ft}", name="o_t")
                nc.vector.tensor_tensor(
                    ob[:],
                    ps,
                    beta_sb[:, ft : ft + 1].to_broadcast((P, HB)),
                    mybir.AluOpType.add,
                )
                nc.vector.tensor_tensor(
                    ob[:], ob[:], upc[:, ft, HB : 2 * HB], mybir.AluOpType.add
                )
                eng = nc.scalar if ft % 2 == 0 else nc.sync
                eng.dma_start(out_r[:, ft, HB : 2 * HB], ob[:])

            drain(z_units(gs0b, 0, 0, HB, fin0b, tail=True))

    nc.compile()
    return nc


# ---------------------------------------------------------------------------
# host-side input preparation / sharding
# ---------------------------------------------------------------------------

def make_in_maps(cfg, query, in_proj_w, in_proj_b, out_w, out_b, up_w, up_b):
    S, E, HD, F, ECH = cfg["S"], cfg["E"], cfg["HD"], cfg["F"], cfg["ECH"]
    FT = ECH
    f32 = np.float32
    f64 = np.float64

    query = np.asarray(query, f32)
    in_proj_w = np.asarray(in_proj_w, f32)
    in_proj_b = np.asarray(in_proj_b, f32)
    out_w = np.asarray(out_w, f32)
    out_b = np.asarray(out_b, f32)
    up_w = np.asarray(up_w, f32)
    up_b = np.asarray(up_b, f32)

    qT = np.ascontiguousarray(query[0].T.astype(BF16_NP))  # [E, S]

    # folded epilogue: D_l = W_out[l]^T @ up_w[l-1]^T @ ... @ up_w[0]^T
    # beta: beta_3 = b3; beta_l = beta_{l+1} @ up_w[l]^T + up_b[l] + b_l
    # V-bias fold: a V bias b_v shifts A_l by b_v (softmax weights sum to 1),
    # and the epilogue is linear, so it contributes b_v @ D_l to beta.
    D = []
    for l in range(LEVELS):
        M = out_w[l].T.astype(f64)
        for j in range(l - 1, -1, -1):
            M = M @ up_w[j].T.astype(f64)
        D.append(M.astype(f32))
    Dm = np.stack(D, axis=0)  # [L, E(in), E(out)] -- already W^T layout
    beta = out_b[3].astype(f64)
    for l in range(LEVELS - 2, -1, -1):
        beta = beta @ up_w[l].T.astype(f64) + up_b[l] + out_b[l]
    for l in range(LEVELS):
        bv = in_proj_b[l, 2 * E : 3 * E].astype(f64)
        beta = beta + bv @ D[l].astype(f64)
    beta = beta.astype(f32)

    # pack [L, e_in, e_out] -> [L, e_in%128, e_in//128, e_out//128, e_out%128]
    t = Dm.reshape(LEVELS, ECH, P, FT, P)          # [L, ec, ep, ft, fp]
    t = t.transpose(0, 2, 1, 3, 4)                 # [L, ep, ec, ft, fp]
    wd = np.ascontiguousarray(t.astype(BF16_NP))
    beta_pk = np.ascontiguousarray(beta.reshape(FT, P).T.astype(f32))  # [P, FT]

    scale = 1.0 / np.sqrt(HD).astype(f32)
    in_maps = []
    for c in range(NCORES):
        r0 = c * F
        sl_q = in_proj_w[:, r0 : r0 + F, :] * scale          # [L, F, E]
        sl_k = in_proj_w[:, E + r0 : E + r0 + F, :]
        sl_v = in_proj_w[:, 2 * E + r0 : 2 * E + r0 + F, :]
        w3 = np.stack([sl_q, sl_k, sl_v], axis=1)            # [L, 3, F, E]
        w3 = w3.transpose(0, 3, 1, 2)                        # [L, E(e), 3, F]
        w3 = w3.reshape(LEVELS, ECH, P, 3, F).transpose(0, 2, 3, 1, 4)
        w3 = np.ascontiguousarray(w3.astype(BF16_NP))        # [L, p, 3, ch, F]

        b_q = in_proj_b[:, r0 : r0 + F] * scale
        b_k = in_proj_b[:, E + r0 : E + r0 + F]
        b_v = np.zeros_like(b_k)  # folded into beta
        b3 = np.stack([b_q, b_k, b_v], axis=1)               # [L, 3, F]
        b3 = np.zeros((P, LEVELS, 3), f32) + b3.transpose(2, 0, 1)

        in_maps.append(
            {
                "qT": qT,
                "win": w3,
                "bin": np.ascontiguousarray(b3),
                "wd": wd,
                "beta": beta_pk,
            }
        )
    return in_maps


def assemble_output(cfg, results):
    S, E = cfg["S"], cfg["E"]
    blk = cfg["BLK"][0]
    out = np.empty((1, S, E), np.float32)
    for c in range(NCORES):
        out[0, c * blk : (c + 1) * blk, :] = results[c]["out"].T
    return out


_CACHE = {}


def _get_nc(cfg_key=(2048, 1024, 16)):
    if cfg_key not in _CACHE:
        cfg = _cfg(*cfg_key)
        _CACHE[cfg_key] = (cfg, build(cfg))
    return _CACHE[cfg_key]


def kernel(query, in_proj_w, in_proj_b, out_w, out_b, up_w, up_b):
    from concourse.bass_utils import run_bass_kernel_spmd

    cfg, nc = _get_nc()
    in_maps = make_in_maps(cfg, query, in_proj_w, in_proj_b, out_w, out_b, up_w, up_b)
    res = run_bass_kernel_spmd(nc, in_maps, core_ids=list(range(NCORES)))
    return assemble_output(cfg, res.results)


# revision 31
# speedup vs baseline: 1.1180x; 1.1180x over previous
"""Trainium2 Bass kernel for AdaptiveHierarchicalAttention (8 NeuronCores).

Reference computation (per level l in 0..3):
    x_l = query[:, ::2^l, :]                         # [1, S_l, E], S_l = S >> l
    outs[l] = MHA_l(x_l)                             # 16-head self-attention
Bottom-up: current = outs[3]; for l in (2,1,0):
    current = upsample_linear(current, S_l) @ up_w[l].T + up_b[l] + outs[l]

Sharding (8 cores): tensor-parallel over heads (2 heads/core) for every
level's attention; the up-propagation chain is folded on the host into one
E x E matrix D_l per level (plus a bias vector beta), so the device epilogue
is Z_l = A_l @ D_l at each level's resolution followed by chained 2x
linear-interp upsampling and adds.

Schedule (the critical resources are the PE (~108us of bf16 matmul) and the
Activation engine (~88us of softmax exp); collectives cost 15us fixed each in
the cost model so their issue order decides the tail):
  - Level order: 3, 2 first (their fused exchange is in flight by ~30us and
    hides under level-0 compute), then level 0 pass A (first 128-token half
    of every dest core's block, q-strided), then level 1, then level 0 pass
    B.  Exchanges fire immediately after their producer: a2a32, a2a0a, a2a1,
    a2a0b.  Only the last one (one half of level 0) is exposed, followed by a
    ~6us epilogue tail.
  - V is produced directly token-major (x-chunk as lhsT against the V weight
    block), eliminating the PE transposes and DVE repacks of the baseline; V
    bias folds into the host-side beta (softmax weights sum to 1, so a V bias
    shifts A by a constant that the linear epilogue maps to a constant).
  - QKV projection and epilogue Z_l matmul work is emitted as queues of
    closures drained into the PE bubbles of attention blocks (where PE waits
    on ScalarE exps), with per-call budgets so the Activation engine never
    starves.
  - Attention math: scoresT[k,q] = K^T Q per head via feature-major Q/K
    straight from the projection; exp on ScalarE without max subtraction
    (scores are O(1)); AV token-major with an appended ones column so the
    softmax denominator falls out of the same matmuls; per-partition
    reciprocal broadcast on DVE; PE transpose back to feature-major.

kernel(**inputs) takes the FULL unsharded inputs and returns the FULL output.
"""

import sys

import numpy as np

sys.path.insert(0, "/opt/trn_rl_repo")

import ml_dtypes  # noqa: E402

import concourse.mybir as mybir  # noqa: E402
import concourse.tile as tile  # noqa: E402
from concourse import bacc  # noqa: E402
from concourse.masks import make_identity  # noqa: E402

F32 = mybir.dt.float32
BF16 = mybir.dt.bfloat16
BF16_NP = ml_dtypes.bfloat16

NCORES = 8
LEVELS = 4
P = 128


def _cfg(S=2048, E=1024, H=16):
    c = {}
    c["S"], c["E"], c["H"] = S, E, H
    c["HD"] = E // H                    # head dim
    c["HPC"] = H // NCORES              # heads per core
    c["F"] = c["HPC"] * c["HD"]         # feature rows per core
    assert c["F"] == 128, "per-core feature slice must be 128"
    c["ECH"] = E // P                   # contraction chunks
    c["SL"] = [S >> l for l in range(LEVELS)]
    c["LOFF"] = np.cumsum([0] + c["SL"]).tolist()   # level offsets in token concat
    c["T"] = sum(c["SL"])               # total tokens across levels
    c["CH"] = [sl // P for sl in c["SL"]]
    c["CHOFF"] = np.cumsum([0] + c["CH"]).tolist()
    c["CHT"] = sum(c["CH"])
    c["BLK"] = [sl // NCORES for sl in c["SL"]]     # per-core token block
    # epilogue windows (token ranges incl. halos): level 0 has no halo.
    c["WIN"] = [c["BLK"][0], c["BLK"][1] + 2, c["BLK"][2] + 4, c["BLK"][3] + 4]
    # upsample phase per step l+1 -> l  (True = "even" pattern A)
    c["PHASE_A"] = [True, False, True]  # index by l of target level 0,1,2
    c["PAD"] = 2
    c["QB0"] = min(512, c["SL"][0])     # level-0 q-block width
    return c


# ---------------------------------------------------------------------------
# builder
# ---------------------------------------------------------------------------

def build(cfg, kgroup=8):
    S, E = cfg["S"], cfg["E"]
    HD, F, ECH = cfg["HD"], cfg["F"], cfg["ECH"]
    SL, LOFF, T = cfg["SL"], cfg["LOFF"], cfg["T"]
    CH, CHOFF, CHT = cfg["CH"], cfg["CHOFF"], cfg["CHT"]
    BLK, WIN, PAD = cfg["BLK"], cfg["WIN"], cfg["PAD"]
    QB0 = cfg["QB0"]
    FT = ECH  # number of 128-wide feature tiles of E
    VW = 2 * HD + 4  # V-token chunk width: [V_A | 1 | pad | V_B | 1 | pad]

    nc = bacc.Bacc(
        "TRN2",
        target_bir_lowering=False,
        debug=False,
        enable_asserts=False,
        num_devices=NCORES,
    )

    # --- I/O ---------------------------------------------------------------
    qT = nc.dram_tensor("qT", [E, S], BF16, kind="ExternalInput")
    win_p = nc.dram_tensor("win", [LEVELS, P, 3, ECH, F], BF16, kind="ExternalInput")
    bin_p = nc.dram_tensor("bin", [P, LEVELS, 3], F32, kind="ExternalInput")
    # folded epilogue weights D_l (E x E each) packed for lhsT use, + beta
    wd_p = nc.dram_tensor("wd", [LEVELS, P, ECH, FT, P], BF16, kind="ExternalInput")
    beta_p = nc.dram_tensor("beta", [P, FT], F32, kind="ExternalInput")
    out_p = nc.dram_tensor("out", [E, BLK[0]], F32, kind="ExternalOutput")

    # --- internal DRAM (collective bounce) ---------------------------------
    CW = [SL[3] + 2 * PAD, SL[2] + 2 * PAD, SL[1] + 2 * PAD]
    CO = {3: 0, 2: CW[0], 1: CW[0] + CW[1]}      # concat offset per level
    CTOT = sum(CW)
    HALO = {1: 1, 2: 2, 3: 2}
    W32 = WIN[3] + WIN[2]
    agin32 = nc.dram_tensor("agin32", [NCORES, P, W32], BF16)
    g32 = nc.dram_tensor("g32", [NCORES, P, W32], BF16)
    agin1 = nc.dram_tensor("agin1", [NCORES, P, WIN[1]], BF16)
    g1 = nc.dram_tensor("g1", [NCORES, P, WIN[1]], BF16)
    HB = BLK[0] // 2
    agin0a = nc.dram_tensor("agin0a", [NCORES, P, HB], BF16)
    g0a = nc.dram_tensor("g0a", [NCORES, P, HB], BF16)
    agin0b = nc.dram_tensor("agin0b", [NCORES, P, HB], BF16)
    g0b = nc.dram_tensor("g0b", [NCORES, P, HB], BF16)
    rg = [list(range(NCORES))]

    with tile.TileContext(nc) as tc:
        from contextlib import ExitStack

        with ExitStack() as ctx:
            pool = lambda name, bufs, **kw: ctx.enter_context(
                tc.tile_pool(name=name, bufs=bufs, **kw)
            )
            const = pool("const", 1)
            d_pool = pool("dw", 1)
            g_pool = pool("gpool", 1)
            qk_pool = pool("qk", 1)
            at_pool = pool("at", 16)
            nrm_pool = pool("nrm", 4)
            a0_pool = pool("a0", 3)
            # QKV weights die after the last projection (end of pass A);
            # their pool closes there and the epilogue acc pool reuses the
            # space.
            wl_stack = ctx.enter_context(ExitStack())
            wl_pool = wl_stack.enter_context(tc.tile_pool(name="wl", bufs=1))
            qkv_ps = pool("qkv_ps", 2, space="PSUM")
            sc_ps = pool("sc_ps", 2, space="PSUM")
            av_ps = pool("av_ps", 1, space="PSUM")

            # --- constants / persistent buffers ---------------------------
            ident = const.tile([P, P], BF16, tag="ident")
            make_identity(nc, ident[:])
            # f32 identity for the f32 norm transposes (dtypes must match)
            ident32 = const.tile([P, P], F32, tag="ident32")
            nc.vector.tensor_copy(out=ident32[:], in_=ident[:])

            # DMA engine split (transfer time occupies the issuing engine's
            # queue in the cost model): xT is spread over the SP, Pool and
            # Activation queues so the full input is resident by ~7us; Pool
            # then takes the folded-weight prefetch, paused around its
            # collectives so they fire at data-ready.
            wl_sb = wl_pool.tile([P, LEVELS, 3, ECH, F], BF16, tag="wl_sb")
            win_r = win_p.ap().rearrange("l p three c f -> p l three c f")
            xT = qk_pool.tile([P, ECH, S], BF16, tag="xT")
            qT_r = qT.ap().rearrange("(c p) t -> p c t", p=P)
            b_sb = const.tile([P, LEVELS, 3], F32, tag="b_sb")
            # SP: bias, level-3 Q/K weights, xT 0-2, then remaining weights
            nc.sync.dma_start(b_sb[:], bin_p[:])
            nc.sync.dma_start(wl_sb[:, 3, 0], win_r[:, 3, 0])
            nc.sync.dma_start(wl_sb[:, 3, 1], win_r[:, 3, 1])
            for c in (0, 1, 2):
                nc.sync.dma_start(xT[:, c, :], qT_r[:, c, :])
            # Activation: xT 6-7 + level-3 V weights, then the exp-table warm
            for c in (6, 7):
                nc.scalar.dma_start(xT[:, c, :], qT_r[:, c, :])
            nc.scalar.dma_start(wl_sb[:, 3, 2], win_r[:, 3, 2])
            warm = const.tile([P, 2], BF16, tag="warm")
            nc.scalar.activation(
                warm[:], ident[:, 0:2], mybir.ActivationFunctionType.Exp
            )
            # Pool: xT 3-5, level-2 weights, then folded weights for 3+2
            for c in (3, 4, 5):
                nc.gpsimd.dma_start(xT[:, c, :], qT_r[:, c, :])
            nc.gpsimd.dma_start(wl_sb[:, 2], win_r[:, 2])
            for l in (1, 0):
                nc.sync.dma_start(wl_sb[:, l], win_r[:, l])
            beta_sb = const.tile([P, FT], F32, tag="beta_sb")
            nc.sync.dma_start(beta_sb[:], beta_p[:])

            wd_sb = d_pool.tile([P, LEVELS, ECH, FT, P], BF16, tag="wd_sb")
            wd_r = wd_p.ap().rearrange("l p c ft f -> p l c ft f")
            for l in (3, 2):
                for c in range(ECH):
                    nc.gpsimd.dma_start(wd_sb[:, l, c], wd_r[:, l, c])

            Q = qk_pool.tile([P, T], BF16, tag="Q")
            K = qk_pool.tile([P, T], BF16, tag="K")
            # level-0 scores run as fp8 DoubleRow: Q8 holds the fp8 query
            # twice (both DR k-tiles), K8 holds {hi, lo = K - hi} so K keeps
            # near-bf16 precision; only Q's fp8 rounding enters the scores.
            F8 = mybir.dt.float8e4
            Q8 = qk_pool.tile([P, 2, SL[0]], F8, tag="Q8")
            K8 = qk_pool.tile([P, 2, SL[0]], F8, tag="K8")
            Vt = qk_pool.tile([P, CHT, VW], BF16, tag="Vt")
            nc.vector.memset(Vt[:, :, HD : HD + 1], 1.0)
            nc.vector.memset(Vt[:, :, 2 * HD + 2 : 2 * HD + 3], 1.0)

            # ---------------- per-level QKV -------------------------------
            # Emitted as queues of closures, drained into PE bubbles of
            # attention blocks.  Q/K are feature-major; V goes directly
            # token-major into Vt (x-chunk as lhsT, V-weight block as rhs),
            # so no V transposes are needed.  V bias is folded into beta on
            # the host.
            def qkv_chunks(l):
                stride = 1 << l
                sl = SL[l]
                nt = min(512, sl)

                def proj(part, n0, dst):
                    def emit():
                        ps = qkv_ps.tile([F, 512], F32, tag="qkv", name="qkvps")
                        for c in range(ECH):
                            rhs = xT[:, c, n0 * stride : (n0 + nt) * stride : stride]
                            nc.tensor.matmul(
                                ps[:, 0:nt],
                                lhsT=wl_sb[:, l, part, c, :],
                                rhs=rhs,
                                start=(c == 0),
                                stop=(c == ECH - 1),
                            )
                        bias = b_sb[:, l, part : part + 1].to_broadcast((F, nt))
                        if l == 0 and part == 0:
                            # fp8 query, duplicated into both DR k-tiles
                            for t8 in (0, 1):
                                nc.vector.tensor_tensor(
                                    Q8[:, t8, n0 : n0 + nt], ps[:, 0:nt], bias,
                                    mybir.AluOpType.add,
                                )
                            return
                        o = dst[:, LOFF[l] + n0 : LOFF[l] + n0 + nt]
                        nc.vector.tensor_tensor(
                            o, ps[:, 0:nt], bias, mybir.AluOpType.add
                        )
                        if l == 0 and part == 1:
                            # fp8 key pair: hi = fp8(K), lo = fp8(K - hi)
                            nc.vector.tensor_copy(
                                out=K8[:, 0, n0 : n0 + nt], in_=o
                            )
                            nc.vector.tensor_tensor(
                                K8[:, 1, n0 : n0 + nt], o, K8[:, 0, n0 : n0 + nt],
                                mybir.AluOpType.subtract,
                            )

                    return emit

                def vproj(j):
                    def emit():
                        ps = qkv_ps.tile([F, 512], F32, tag="qkv", name="qkvps")
                        out = ps[:, 0:P]          # [128 tok, 128 dim]
                        for c in range(ECH):
                            lhsT = xT[:, c, j * P * stride : (j + 1) * P * stride : stride]
                            nc.tensor.matmul(
                                out,
                                lhsT=lhsT,
                                rhs=wl_sb[:, l, 2, c, :],
                                start=(c == 0),
                                stop=(c == ECH - 1),
                            )
                        src = ps[:, 0:HD]
                        src.ap.insert(1, [HD, 2])
                        ch = CHOFF[l] + j
                        dstv = Vt[:, ch, 0:HD]
                        dstv.ap.insert(1, [HD + 2, 2])
                        nc.vector.tensor_copy(out=dstv, in_=src)

                    return emit

                work = []
                for part, dst in ((0, Q), (1, K)):
                    for n0 in range(0, sl, nt):
                        work.append(proj(part, n0, dst))
                for j in range(CH[l]):
                    work.append(vproj(j))
                return work

            def mk_filler(work, schedule):
                """Filler draining `work` with a per-call budget schedule."""
                state = {"i": 0}

                def filler():
                    k = (
                        schedule[state["i"]]
                        if state["i"] < len(schedule)
                        else (schedule[-1] if schedule else 0)
                    )
                    state["i"] += 1
                    for _ in range(min(k, len(work))):
                        work.pop(0)()

                return filler

            def drain(work):
                while work:
                    work.pop(0)()

            def attn_block(l, qb0, qbw, a_dst, a_off, filler=None, qstride=None):
                """Attention for q-block [qb0, qb0+qbw) of level l -> a_dst[:, a_off:].

                With qstride, the q-set is qbw//128 chunks of 128 tokens at
                stride qstride starting at qb0 (strided level-0 half-blocks).

                Emission interleaves score/exp units with the AV matmuls of
                the unit two steps back (matching the 2-buffer sc_ps
                rotation), so PE trails the exp stream by ~2us with no
                stall cascade.  The filler is called once before the first
                AV (for work that must precede AVs, e.g. V projections) and
                once after each AV group.
                """
                nch = CH[l]
                nqc = qbw // P
                fp8 = l == 0  # scores via fp8 DoubleRow (Q8 x (K_hi+K_lo))

                def qrhs(b, qc=None):
                    if qstride is None:
                        return Q[b : b + HD, LOFF[l] + qb0 : LOFF[l] + qb0 + qbw]
                    if qc is None:
                        r = Q[b : b + HD, LOFF[l] + qb0 : LOFF[l] + qb0 + P]
                        r.ap.insert(1, [qstride, nqc])
                        return r
                    # fp8 DoubleRow path: one contiguous 128-token q-chunk
                    q0 = qb0 + qc * qstride
                    return Q8[b : b + HD, :, q0 : q0 + P]

                avb = av_ps.tile([P, 8, P], F32, tag="avb")
                ats = {}
                seq = []
                for i0 in range(0, nch, 2):
                    for h in (0, 1):
                        seq.append((list(range(i0, min(i0 + 2, nch))), h))

                def emit_sc(pair, h):
                    b = h * HD
                    sp = sc_ps.tile([P, 2 * qbw], F32, tag="sc")
                    for j, kc in enumerate(pair):
                        if fp8:
                            for qc in range(nqc):
                                nc.tensor.matmul(
                                    sp[:, j * qbw + qc * P : j * qbw + (qc + 1) * P],
                                    lhsT=K8[b : b + HD, :, kc * P : (kc + 1) * P],
                                    rhs=qrhs(b, qc),
                                    start=True,
                                    stop=True,
                                    perf_mode=mybir.MatmulPerfMode.DoubleRow,
                                )
                        else:
                            nc.tensor.matmul(
                                sp[:, j * qbw : (j + 1) * qbw],
                                lhsT=K[b : b + HD, LOFF[l] + kc * P : LOFF[l] + (kc + 1) * P],
                                rhs=qrhs(b),
                                start=True,
                                stop=True,
                            )
                    at = at_pool.tile([P, 2 * qbw], BF16, tag="at")
                    nc.scalar.activation(
                        at[:, 0 : len(pair) * qbw],
                        sp[:, 0 : len(pair) * qbw],
                        mybir.ActivationFunctionType.Exp,
                    )
                    for j, kc in enumerate(pair):
                        ats[(kc, h)] = at[:, j * qbw : (j + 1) * qbw]

                def emit_av(pair, h):
                    for kc in pair:
                        for qc in range(nqc):
                            slot = qc * 2 + h
                            c0 = 0 if h == 0 else HD + 2
                            # one start per 2KB PSUM bank: the first write
                            # marks the whole bank pending-zero, the other
                            # slots' first writes self-zero.  stop on the
                            # chronologically last write to the bank.
                            nc.tensor.matmul(
                                avb[:, slot, 0 : HD + 1],
                                lhsT=ats[(kc, h)][:, qc * P : (qc + 1) * P],
                                rhs=Vt[:, CHOFF[l] + kc, c0 : c0 + HD + 1],
                                start=(kc == 0 and h == 0 and qc % 2 == 0),
                                stop=(
                                    kc == nch - 1
                                    and h == 1
                                    and (qc % 2 == 1 or qc == nqc - 1)
                                ),
                            )

                for i, (pair, h) in enumerate(seq):
                    emit_sc(pair, h)
                    if i == 1 and filler is not None:
                        filler()
                    if i >= 2:
                        emit_av(*seq[i - 2])
                        if filler is not None:
                            filler()
                for i in range(max(0, len(seq) - 2), len(seq)):
                    emit_av(*seq[i])
                    if filler is not None:
                        filler()

                # normalize (per-partition reciprocal of the denominator
                # column), pack both heads side by side, transpose back to
                # feature-major via the PE, copy into the destination buffer
                for qc in range(nqc):
                    sb = nrm_pool.tile([P, P], F32, tag="sb")
                    for h in (0, 1):
                        slot = qc * 2 + h
                        r = nrm_pool.tile([P, 1], F32, tag=f"r{h}", name="rcp")
                        nc.vector.reciprocal(
                            r[:, 0:1], avb[:, slot, HD : HD + 1]
                        )
                        nc.vector.tensor_mul(
                            out=sb[:, h * HD : (h + 1) * HD],
                            in0=avb[:, slot, 0:HD],
                            in1=r[:, 0:1].to_broadcast((P, HD)),
                        )
                    # f32 transpose through the 2-buffer qkv PSUM pool so
                    # consecutive qc transposes pipeline with the copies
                    tp = qkv_ps.tile([F, 512], F32, tag="qkv", name="trp")
                    nc.tensor.transpose(tp[:, 0:P], sb[:], ident32[:])
                    nc.vector.tensor_copy(
                        out=a_dst[:, a_off + qc * P : a_off + (qc + 1) * P],
                        in_=tp[:, 0:P],
                    )

            A123 = qk_pool.tile([P, CTOT], BF16, tag="A123")

            def pad_edges(l):
                sl = SL[l]
                co = CO[l]
                nc.vector.tensor_copy(
                    out=A123[:, co : co + PAD],
                    in_=A123[:, co + PAD : co + PAD + 1].to_broadcast((P, PAD)),
                )
                nc.vector.tensor_copy(
                    out=A123[:, co + PAD + sl : co + 2 * PAD + sl],
                    in_=A123[:, co + PAD + sl - 1 : co + PAD + sl].to_broadcast((P, PAD)),
                )

            def attn_level_whole(l, filler):
                sl = SL[l]
                co = CO[l]
                qbw = min(512, sl)
                for qb0 in range(0, sl, qbw):
                    attn_block(l, qb0, qbw, A123, co + PAD + qb0, filler=filler)
                pad_edges(l)

            def bounce_windows(l, dst_dram, woff):
                """One DMA materializing all 8 overlapping dest windows."""
                s0 = CO[l] + PAD - HALO[l]
                src = A123[:, s0 : s0 + WIN[l]]
                src.ap.insert(1, [BLK[l], NCORES])
                dst = dst_dram.ap().rearrange("d p w -> p d w")
                nc.sync.dma_start(dst[:, :, woff : woff + WIN[l]], src)

            def a2a(ins_t, outs_t):
                nc.gpsimd.collective_compute(
                    "AllToAll",
                    mybir.AluOpType.bypass,
                    replica_groups=rg,
                    ins=[ins_t[:]],
                    outs=[outs_t[:]],
                )

            def attn_level0_pass(filler, half, ag, gout):
                """One strided half-pass of level 0: the q-set is the
                `half`-th 128-token half of every dest core's 256-block."""
                for b in range(2):
                    A0 = a0_pool.tile([P, QB0], BF16, tag="A0")
                    attn_block(
                        0,
                        b * 4 * BLK[0] + half * HB,
                        QB0,
                        A0,
                        0,
                        filler=filler,
                        qstride=BLK[0],
                    )
                    # 4 dests x 128 cols per block
                    nc.sync.dma_start(
                        ag.ap()[b * 4 : (b + 1) * 4].rearrange("d p w -> p d w"),
                        A0[:].rearrange("p (d w) -> p d w", d=4),
                    )
                a2a(ag, gout)

            # ---------------- epilogue work units -------------------------
            # Z_l = (gathered A_l window) @ D_l at level-l resolution, per
            # 128-wide output feature tile; emitted as PE filler closures.
            def z_units(gtile, goff, l, w, fin, tail=False):
                """One closure per ft: matmul into psum, then fin(ft, ps).

                tail=True uses the (then idle) 2-buffer score PSUM pool so
                consecutive units pipeline instead of ping-ponging on the
                single qkv bank.
                """
                units = []
                for ft in range(FT):
                    def emit(ft=ft):
                        if tail:
                            ps = sc_ps.tile([P, 1024], F32, tag="sc", name="zps")
                        else:
                            ps = qkv_ps.tile([F, 512], F32, tag="qkv", name="qkvps")
                        for c in range(ECH):
                            nc.tensor.matmul(
                                ps[:, 0:w],
                                lhsT=wd_sb[:, l, c, ft],
                                rhs=gtile[:, c, goff : goff + w],
                                start=(c == 0),
                                stop=(c == ECH - 1),
                            )
                        fin(ft, ps[:, 0:w])
                    units.append(emit)
                return units

            def upsample(cur, ws, w, phase_a, tag):
                """2x linear-interp upsample [P, FT, ws] -> [P, FT, w] (DVE)."""
                p25 = acc_pool.tile([P, FT, ws], BF16, tag=f"p25{tag}")
                p75 = acc_pool.tile([P, FT, ws], BF16, tag=f"p75{tag}")
                nc.vector.tensor_scalar_mul(p25[:], cur[:], 0.25)
                nc.vector.tensor_scalar_mul(p75[:], cur[:], 0.75)
                up = acc_pool.tile([P, FT, w], BF16, tag=f"up{tag}")
                hw = (w + 1) // 2
                hw2 = w // 2
                if phase_a:
                    nc.vector.tensor_add(
                        up[:, :, 0::2], p25[:, :, 0:hw], p75[:, :, 1 : hw + 1]
                    )
                    nc.vector.tensor_add(
                        up[:, :, 1::2], p75[:, :, 1 : hw2 + 1], p25[:, :, 2 : hw2 + 2]
                    )
                else:
                    nc.vector.tensor_add(
                        up[:, :, 0::2], p75[:, :, 1 : hw + 1], p25[:, :, 2 : hw + 2]
                    )
                    nc.vector.tensor_add(
                        up[:, :, 1::2], p25[:, :, 1 : hw2 + 1], p75[:, :, 2 : hw2 + 2]
                    )
                return up

            # ---------------- schedule ------------------------------------
            # Attention order: 3, 2, 1, 0A, 0B.  Exchange order: a2a32,
            # a2a1, a2a0a, a2a0b -- each fires at data-ready into a free
            # slot on the Pool queue; only a2a0b is exposed at the tail.
            w3 = qkv_chunks(3)
            w2 = qkv_chunks(2)
            w1 = qkv_chunks(1)
            w0 = qkv_chunks(0)
            w1qk, w1v = w1[:4], w1[4:]
            w0qk, w0v = w0[:8], w0[8:]

            drain(w3)
            # level 3 attention; fill with level-2 QKV (6 units)
            attn_level_whole(3, mk_filler(w2, [2, 2, 2]))
            drain(w2)
            bounce_windows(3, agin32, 0)
            # level 2 attention; fill with level-1 Q/K
            attn_level_whole(2, mk_filler(w1qk, [0, 1, 1, 1, 1]))
            drain(w1qk)
            bounce_windows(2, agin32, WIN[3])
            a2a(agin32, g32)
            Gs32 = g_pool.tile([P, ECH, W32], BF16, tag="gs32")
            nc.sync.dma_start(Gs32[:], g32.ap().rearrange("b p t -> p b t"))

            # level 1 attention: V of L1 in the pre-AV slot (needed before
            # its AVs), then level-0 Q/K and level-0 V one unit per AV slot.
            wB = w1v + w0qk + w0v
            fB = mk_filler(wB, [8] + [1] * 30)
            attn_block(1, 0, 512, A123, CO[1] + PAD, filler=fB)
            attn_block(1, 512, 512, A123, CO[1] + PAD + 512, filler=fB)
            drain(wB)
            pad_edges(1)
            bounce_windows(1, agin1, 0)
            a2a(agin1, g1)
            Gs1 = g_pool.tile([P, ECH, WIN[1]], BF16, tag="gs1")
            nc.sync.dma_start(Gs1[:], g1.ap().rearrange("b p t -> p b t"))
            # folded weights for levels 1, 0 prefetch behind a2a1 on Pool
            for l in (1, 0):
                for c in range(ECH):
                    nc.gpsimd.dma_start(wd_sb[:, l, c], wd_r[:, l, c])

            # level 0 pass A; Z3 epilogue matmuls spread over the second
            # block's AV slots (Gs32 has landed by then).
            zt3 = g_pool.tile([P, FT, WIN[3]], BF16, tag="zt3")

            def fin3(ft, ps):
                nc.vector.tensor_copy(out=zt3[:, ft, :], in_=ps)

            z3u = z_units(Gs32, 0, 3, WIN[3], fin3)
            attn_level0_pass(mk_filler(z3u, [0] * 18 + [1] * 16), 0, agin0a, g0a)
            drain(z3u)
            wl_stack.close()
            acc_pool = pool("acc", 1)
            Gs0a = g_pool.tile([P, ECH, HB], BF16, tag="gs0a")
            nc.sync.dma_start(Gs0a[:], g0a.ap().rearrange("b p t -> p b t"))
            up_a = upsample(zt3, WIN[3], WIN[2], cfg["PHASE_A"][2], "a")

            # level 0 pass B with the Z2 then (up_b) then Z1 epilogue as
            # fillers; Gs1 lands early in this window.
            acc2 = acc_pool.tile([P, FT, WIN[2]], BF16, tag="acc2")
            acc1 = acc_pool.tile([P, FT, WIN[1]], BF16, tag="acc1")

            def fin2(ft, ps):
                nc.vector.tensor_tensor(
                    acc2[:, ft, :], ps, up_a[:, ft, :], mybir.AluOpType.add
                )

            def fin1(ft, ps):
                nc.vector.tensor_tensor(
                    acc1[:, ft, :], ps, up_b[:, ft, :], mybir.AluOpType.add
                )

            z2u = z_units(Gs32, WIN[3], 2, WIN[2], fin2)
            up_b = None

            def emit_up_b():
                nonlocal up_b
                up_b = upsample(acc2, WIN[2], WIN[1], cfg["PHASE_A"][1], "b")

            z1u = z_units(Gs1, 0, 1, WIN[1], fin1)
            wC = z2u + [emit_up_b] + z1u
            attn_level0_pass(mk_filler(wC, [0] + [1] * 40), 1, agin0b, g0b)
            drain(wC)

            # half-A epilogue + chain to upc while the last exchange flies
            o_a = acc_pool.tile([P, FT, HB], F32, tag="o_a")

            def fin0a(ft, ps):
                nc.vector.tensor_tensor(
                    o_a[:, ft, :],
                    ps,
                    beta_sb[:, ft : ft + 1].to_broadcast((P, HB)),
                    mybir.AluOpType.add,
                )

            drain(z_units(Gs0a, 0, 0, HB, fin0a, tail=True))
            upc = upsample(acc1, WIN[1], WIN[0], cfg["PHASE_A"][0], "c")

            # output half A: final add + streamed per-ft DMA.  The output
            # DMAs go on the Activation queue (its exps are done by now),
            # keeping SP free and the tail off the critical path.
            out_r = out_p.ap().rearrange("(c p) t -> p c t", p=P)
            for ft in range(FT):
                oa = acc_pool.tile([P, HB], F32, tag=f"oA{ft}", name="o_t")
                nc.vector.tensor_tensor(
                    oa[:], o_a[:, ft, :], upc[:, ft, 0:HB], mybir.AluOpType.add
                )
                nc.scalar.dma_start(out_r[:, ft, 0:HB], oa[:])

            # tail: half B (needs the last exchange); load split over two
            # DMA queues so it lands in ~0.8us.
            gs0b = g_pool.tile([P, ECH, HB], BF16, tag="gs0b")
            g0b_r = g0b.ap().rearrange("b p t -> p b t")
            nc.sync.dma_start(gs0b[:, 0:4], g0b_r[:, 0:4])
            nc.scalar.dma_start(gs0b[:, 4:8], g0b_r[:, 4:8])

            def fin0b(ft, ps):
                ob = acc_pool.tile([P, HB], F32, tag=f"oB{ft}", name="o_t")
                nc.vector.tensor_tensor(
                    ob[:],
                    ps,
                    beta_sb[:, ft : ft + 1].to_broadcast((P, HB)),
                    mybir.AluOpType.add,
                )
                nc.vector.tensor_tensor(
                    ob[:], ob[:], upc[:, ft, HB : 2 * HB], mybir.AluOpType.add
                )
                eng = nc.scalar if ft % 2 == 0 else nc.sync
                eng.dma_start(out_r[:, ft, HB : 2 * HB], ob[:])

            drain(z_units(gs0b, 0, 0, HB, fin0b, tail=True))

    nc.compile()
    return nc


# ---------------------------------------------------------------------------
# host-side input preparation / sharding
# ---------------------------------------------------------------------------

def make_in_maps(cfg, query, in_proj_w, in_proj_b, out_w, out_b, up_w, up_b):
    S, E, HD, F, ECH = cfg["S"], cfg["E"], cfg["HD"], cfg["F"], cfg["ECH"]
    FT = ECH
    f32 = np.float32
    f64 = np.float64

    query = np.asarray(query, f32)
    in_proj_w = np.asarray(in_proj_w, f32)
    in_proj_b = np.asarray(in_proj_b, f32)
    out_w = np.asarray(out_w, f32)
    out_b = np.asarray(out_b, f32)
    up_w = np.asarray(up_w, f32)
    up_b = np.asarray(up_b, f32)

    qT = np.ascontiguousarray(query[0].T.astype(BF16_NP))  # [E, S]

    # folded epilogue: D_l = W_out[l]^T @ up_w[l-1]^T @ ... @ up_w[0]^T
    # beta: beta_3 = b3; beta_l = beta_{l+1} @ up_w[l]^T + up_b[l] + b_l
    # V-bias fold: a V bias b_v shifts A_l by b_v (softmax weights sum to 1),
    # and the epilogue is linear, so it contributes b_v @ D_l to beta.
    D = []
    for l in range(LEVELS):
        M = out_w[l].T.astype(f64)
        for j in range(l - 1, -1, -1):
            M = M @ up_w[j].T.astype(f64)
        D.append(M.astype(f32))
    Dm = np.stack(D, axis=0)  # [L, E(in), E(out)] -- already W^T layout
    beta = out_b[3].astype(f64)
    for l in range(LEVELS - 2, -1, -1):
        beta = beta @ up_w[l].T.astype(f64) + up_b[l] + out_b[l]
    for l in range(LEVELS):
        bv = in_proj_b[l, 2 * E : 3 * E].astype(f64)
        beta = beta + bv @ D[l].astype(f64)
    beta = beta.astype(f32)

    # pack [L, e_in, e_out] -> [L, e_in%128, e_in//128, e_out//128, e_out%128]
    t = Dm.reshape(LEVELS, ECH, P, FT, P)          # [L, ec, ep, ft, fp]
    t = t.transpose(0, 2, 1, 3, 4)                 # [L, ep, ec, ft, fp]
    wd = np.ascontiguousarray(t.astype(BF16_NP))
    beta_pk = np.ascontiguousarray(beta.reshape(FT, P).T.astype(f32))  # [P, FT]

    scale = 1.0 / np.sqrt(HD).astype(f32)
    in_maps = []
    for c in range(NCORES):
        r0 = c * F
        sl_q = in_proj_w[:, r0 : r0 + F, :] * scale          # [L, F, E]
        sl_k = in_proj_w[:, E + r0 : E + r0 + F, :]
        sl_v = in_proj_w[:, 2 * E + r0 : 2 * E + r0 + F, :]
        w3 = np.stack([sl_q, sl_k, sl_v], axis=1)            # [L, 3, F, E]
        w3 = w3.transpose(0, 3, 1, 2)                        # [L, E(e), 3, F]
        w3 = w3.reshape(LEVELS, ECH, P, 3, F).transpose(0, 2, 3, 1, 4)
        w3 = np.ascontiguousarray(w3.astype(BF16_NP))        # [L, p, 3, ch, F]

        b_q = in_proj_b[:, r0 : r0 + F] * scale
        b_k = in_proj_b[:, E + r0 : E + r0 + F]
        b_v = np.zeros_like(b_k)  # folded into beta
        b3 = np.stack([b_q, b_k, b_v], axis=1)               # [L, 3, F]
        b3 = np.zeros((P, LEVELS, 3), f32) + b3.transpose(2, 0, 1)

        in_maps.append(
            {
                "qT": qT,
                "win": w3,
                "bin": np.ascontiguousarray(b3),
                "wd": wd,
                "beta": beta_pk,
            }
        )
    return in_maps


def assemble_output(cfg, results):
    S, E = cfg["S"], cfg["E"]
    blk = cfg["BLK"][0]
    out = np.empty((1, S, E), np.float32)
    for c in range(NCORES):
        out[0, c * blk : (c + 1) * blk, :] = results[c]["out"].T
    return out


_CACHE = {}


def _get_nc(cfg_key=(2048, 1024, 16)):
    if cfg_key not in _CACHE:
        cfg = _cfg(*cfg_key)
        _CACHE[cfg_key] = (cfg, build(cfg))
    return _CACHE[cfg_key]


def kernel(query, in_proj_w, in_proj_b, out_w, out_b, up_w, up_b):
    from concourse.bass_utils import run_bass_kernel_spmd

    cfg, nc = _get_nc()
    in_maps = make_in_maps(cfg, query, in_proj_w, in_proj_b, out_w, out_b, up_w, up_b)
    res = run_bass_kernel_spmd(nc, in_maps, core_ids=list(range(NCORES)))
    return assemble_output(cfg, res.results)


# revision 33
# speedup vs baseline: 1.1491x; 1.0279x over previous
"""Trainium2 Bass kernel for AdaptiveHierarchicalAttention (8 NeuronCores).

Reference computation (per level l in 0..3):
    x_l = query[:, ::2^l, :]                         # [1, S_l, E], S_l = S >> l
    outs[l] = MHA_l(x_l)                             # 16-head self-attention
Bottom-up: current = outs[3]; for l in (2,1,0):
    current = upsample_linear(current, S_l) @ up_w[l].T + up_b[l] + outs[l]

Sharding (8 cores): tensor-parallel over heads (2 heads/core) for every
level's attention; the up-propagation chain is folded on the host into one
E x E matrix D_l per level (plus a bias vector beta), so the device epilogue
is Z_l = A_l @ D_l at each level's resolution followed by chained 2x
linear-interp upsampling and adds.

Schedule (the critical resources are the PE (~108us of bf16 matmul) and the
Activation engine (~88us of softmax exp); collectives cost 15us fixed each in
the cost model so their issue order decides the tail):
  - Level order: 3, 2 first (their fused exchange is in flight by ~30us and
    hides under level-0 compute), then level 0 pass A (first 128-token half
    of every dest core's block, q-strided), then level 1, then level 0 pass
    B.  Exchanges fire immediately after their producer: a2a32, a2a0a, a2a1,
    a2a0b.  Only the last one (one half of level 0) is exposed, followed by a
    ~6us epilogue tail.
  - V is produced directly token-major (x-chunk as lhsT against the V weight
    block), eliminating the PE transposes and DVE repacks of the baseline; V
    bias folds into the host-side beta (softmax weights sum to 1, so a V bias
    shifts A by a constant that the linear epilogue maps to a constant).
  - QKV projection and epilogue Z_l matmul work is emitted as queues of
    closures drained into the PE bubbles of attention blocks (where PE waits
    on ScalarE exps), with per-call budgets so the Activation engine never
    starves.
  - Attention math: scoresT[k,q] = K^T Q per head via feature-major Q/K
    straight from the projection; exp on ScalarE without max subtraction
    (scores are O(1)); AV token-major with an appended ones column so the
    softmax denominator falls out of the same matmuls; per-partition
    reciprocal broadcast on DVE; PE transpose back to feature-major.

kernel(**inputs) takes the FULL unsharded inputs and returns the FULL output.
"""

import sys

import numpy as np

sys.path.insert(0, "/opt/trn_rl_repo")

import ml_dtypes  # noqa: E402

import concourse.mybir as mybir  # noqa: E402
import concourse.tile as tile  # noqa: E402
from concourse import bacc  # noqa: E402
from concourse.masks import make_identity  # noqa: E402

F32 = mybir.dt.float32
BF16 = mybir.dt.bfloat16
BF16_NP = ml_dtypes.bfloat16

NCORES = 8
LEVELS = 4
P = 128


def _cfg(S=2048, E=1024, H=16):
    c = {}
    c["S"], c["E"], c["H"] = S, E, H
    c["HD"] = E // H                    # head dim
    c["HPC"] = H // NCORES              # heads per core
    c["F"] = c["HPC"] * c["HD"]         # feature rows per core
    assert c["F"] == 128, "per-core feature slice must be 128"
    c["ECH"] = E // P                   # contraction chunks
    c["SL"] = [S >> l for l in range(LEVELS)]
    c["LOFF"] = np.cumsum([0] + c["SL"]).tolist()   # level offsets in token concat
    c["T"] = sum(c["SL"])               # total tokens across levels
    c["CH"] = [sl // P for sl in c["SL"]]
    c["CHOFF"] = np.cumsum([0] + c["CH"]).tolist()
    c["CHT"] = sum(c["CH"])
    c["BLK"] = [sl // NCORES for sl in c["SL"]]     # per-core token block
    # epilogue windows (token ranges incl. halos): level 0 has no halo.
    c["WIN"] = [c["BLK"][0], c["BLK"][1] + 2, c["BLK"][2] + 4, c["BLK"][3] + 4]
    # upsample phase per step l+1 -> l  (True = "even" pattern A)
    c["PHASE_A"] = [True, False, True]  # index by l of target level 0,1,2
    c["PAD"] = 2
    c["QB0"] = min(512, c["SL"][0])     # level-0 q-block width
    return c


# ---------------------------------------------------------------------------
# builder
# ---------------------------------------------------------------------------

def build(cfg, kgroup=8):
    S, E = cfg["S"], cfg["E"]
    HD, F, ECH = cfg["HD"], cfg["F"], cfg["ECH"]
    SL, LOFF, T = cfg["SL"], cfg["LOFF"], cfg["T"]
    CH, CHOFF, CHT = cfg["CH"], cfg["CHOFF"], cfg["CHT"]
    BLK, WIN, PAD = cfg["BLK"], cfg["WIN"], cfg["PAD"]
    QB0 = cfg["QB0"]
    FT = ECH  # number of 128-wide feature tiles of E
    VW = 2 * HD + 4  # V-token chunk width: [V_A | 1 | pad | V_B | 1 | pad]

    nc = bacc.Bacc(
        "TRN2",
        target_bir_lowering=False,
        debug=False,
        enable_asserts=False,
        num_devices=NCORES,
    )

    # --- I/O ---------------------------------------------------------------
    qT = nc.dram_tensor("qT", [E, S], BF16, kind="ExternalInput")
    win_p = nc.dram_tensor("win", [LEVELS, P, 3, ECH, F], BF16, kind="ExternalInput")
    bin_p = nc.dram_tensor("bin", [P, LEVELS, 3], F32, kind="ExternalInput")
    # folded epilogue weights D_l (E x E each) packed for lhsT use, + beta
    wd_p = nc.dram_tensor("wd", [LEVELS, P, ECH, FT, P], BF16, kind="ExternalInput")
    beta_p = nc.dram_tensor("beta", [P, FT], F32, kind="ExternalInput")
    out_p = nc.dram_tensor("out", [E, BLK[0]], F32, kind="ExternalOutput")

    # --- internal DRAM (collective bounce) ---------------------------------
    CW = [SL[3] + 2 * PAD, SL[2] + 2 * PAD, SL[1] + 2 * PAD]
    CO = {3: 0, 2: CW[0], 1: CW[0] + CW[1]}      # concat offset per level
    CTOT = sum(CW)
    HALO = {1: 1, 2: 2, 3: 2}
    W32 = WIN[3] + WIN[2]
    agin32 = nc.dram_tensor("agin32", [NCORES, P, W32], BF16)
    g32 = nc.dram_tensor("g32", [NCORES, P, W32], BF16)
    agin1 = nc.dram_tensor("agin1", [NCORES, P, WIN[1]], BF16)
    g1 = nc.dram_tensor("g1", [NCORES, P, WIN[1]], BF16)
    HB = BLK[0] // 2
    agin0a = nc.dram_tensor("agin0a", [NCORES, P, HB], BF16)
    g0a = nc.dram_tensor("g0a", [NCORES, P, HB], BF16)
    agin0b = nc.dram_tensor("agin0b", [NCORES, P, HB], BF16)
    g0b = nc.dram_tensor("g0b", [NCORES, P, HB], BF16)
    rg = [list(range(NCORES))]

    with tile.TileContext(nc) as tc:
        from contextlib import ExitStack

        with ExitStack() as ctx:
            pool = lambda name, bufs, **kw: ctx.enter_context(
                tc.tile_pool(name=name, bufs=bufs, **kw)
            )
            const = pool("const", 1)
            d_pool = pool("dw", 1)
            g_pool = pool("gpool", 1)
            qk_pool = pool("qk", 1)
            at_pool = pool("at", 16)
            nrm_pool = pool("nrm", 4)
            a0_pool = pool("a0", 3)
            # QKV weights die after the last projection (end of pass A);
            # their pool closes there and the epilogue acc pool reuses the
            # space.
            wl_stack = ctx.enter_context(ExitStack())
            wl_pool = wl_stack.enter_context(tc.tile_pool(name="wl", bufs=1))
            qkv_ps = pool("qkv_ps", 1, space="PSUM")
            tr_ps = pool("tr_ps", 1, space="PSUM")
            sc_ps = pool("sc_ps", 2, space="PSUM")
            av_ps = pool("av_ps", 1, space="PSUM")

            # --- constants / persistent buffers ---------------------------
            ident = const.tile([P, P], BF16, tag="ident")
            make_identity(nc, ident[:])

            # DMA engine split (transfer time occupies the issuing engine's
            # queue in the cost model): xT is spread over the SP, Pool and
            # Activation queues so the full input is resident by ~7us; Pool
            # then takes the folded-weight prefetch, paused around its
            # collectives so they fire at data-ready.
            wl_sb = wl_pool.tile([P, LEVELS, 3, ECH, F], BF16, tag="wl_sb")
            win_r = win_p.ap().rearrange("l p three c f -> p l three c f")
            xT = qk_pool.tile([P, ECH, S], BF16, tag="xT")
            qT_r = qT.ap().rearrange("(c p) t -> p c t", p=P)
            b_sb = const.tile([P, LEVELS, 3], F32, tag="b_sb")
            # SP: bias, level-3 Q/K weights, xT 0-2, then remaining weights
            nc.sync.dma_start(b_sb[:], bin_p[:])
            nc.sync.dma_start(wl_sb[:, 3, 0], win_r[:, 3, 0])
            nc.sync.dma_start(wl_sb[:, 3, 1], win_r[:, 3, 1])
            for c in (0, 1, 2):
                nc.sync.dma_start(xT[:, c, :], qT_r[:, c, :])
            # Activation: xT 6-7 + level-3 V weights, then the exp-table warm
            for c in (6, 7):
                nc.scalar.dma_start(xT[:, c, :], qT_r[:, c, :])
            nc.scalar.dma_start(wl_sb[:, 3, 2], win_r[:, 3, 2])
            warm = const.tile([P, 2], BF16, tag="warm")
            nc.scalar.activation(
                warm[:], ident[:, 0:2], mybir.ActivationFunctionType.Exp
            )
            # Pool: xT 3-5, level-2 weights, then folded weights for 3+2
            for c in (3, 4, 5):
                nc.gpsimd.dma_start(xT[:, c, :], qT_r[:, c, :])
            nc.gpsimd.dma_start(wl_sb[:, 2], win_r[:, 2])
            for l in (1, 0):
                nc.sync.dma_start(wl_sb[:, l], win_r[:, l])
            beta_sb = const.tile([P, FT], F32, tag="beta_sb")
            nc.sync.dma_start(beta_sb[:], beta_p[:])

            wd_sb = d_pool.tile([P, LEVELS, ECH, FT, P], BF16, tag="wd_sb")
            wd_r = wd_p.ap().rearrange("l p c ft f -> p l c ft f")
            for l in (3, 2):
                for c in range(ECH):
                    nc.gpsimd.dma_start(wd_sb[:, l, c], wd_r[:, l, c])

            Q = qk_pool.tile([P, T], BF16, tag="Q")
            K = qk_pool.tile([P, T], BF16, tag="K")
            # level-0 scores run as fp8 DoubleRow: Q8 holds the fp8 query
            # twice (both DR k-tiles), K8 holds {hi, lo = K - hi} so K keeps
            # near-bf16 precision; only Q's fp8 rounding enters the scores.
            F8 = mybir.dt.float8e4
            Q8 = qk_pool.tile([P, 2, SL[0]], F8, tag="Q8")
            K8 = qk_pool.tile([P, 2, SL[0]], F8, tag="K8")
            Vt = qk_pool.tile([P, CHT, VW], BF16, tag="Vt")
            nc.vector.memset(Vt[:, :, HD : HD + 1], 1.0)
            nc.vector.memset(Vt[:, :, 2 * HD + 2 : 2 * HD + 3], 1.0)

            # ---------------- per-level QKV -------------------------------
            # Emitted as queues of closures, drained into PE bubbles of
            # attention blocks.  Q/K are feature-major; V goes directly
            # token-major into Vt (x-chunk as lhsT, V-weight block as rhs),
            # so no V transposes are needed.  V bias is folded into beta on
            # the host.
            def qkv_chunks(l):
                stride = 1 << l
                sl = SL[l]
                nt = min(512, sl)

                def proj(part, n0, dst):
                    def emit():
                        ps = qkv_ps.tile([F, 512], F32, tag="qkv", name="qkvps")
                        for c in range(ECH):
                            rhs = xT[:, c, n0 * stride : (n0 + nt) * stride : stride]
                            nc.tensor.matmul(
                                ps[:, 0:nt],
                                lhsT=wl_sb[:, l, part, c, :],
                                rhs=rhs,
                                start=(c == 0),
                                stop=(c == ECH - 1),
                            )
                        bias = b_sb[:, l, part : part + 1].to_broadcast((F, nt))
                        if l == 0 and part == 0:
                            # fp8 query, duplicated into both DR k-tiles
                            for t8 in (0, 1):
                                nc.vector.tensor_tensor(
                                    Q8[:, t8, n0 : n0 + nt], ps[:, 0:nt], bias,
                                    mybir.AluOpType.add,
                                )
                            return
                        o = dst[:, LOFF[l] + n0 : LOFF[l] + n0 + nt]
                        nc.vector.tensor_tensor(
                            o, ps[:, 0:nt], bias, mybir.AluOpType.add
                        )
                        if l == 0 and part == 1:
                            # fp8 key pair: hi = fp8(K), lo = fp8(K - hi)
                            nc.vector.tensor_copy(
                                out=K8[:, 0, n0 : n0 + nt], in_=o
                            )
                            nc.vector.tensor_tensor(
                                K8[:, 1, n0 : n0 + nt], o, K8[:, 0, n0 : n0 + nt],
                                mybir.AluOpType.subtract,
                            )

                    return emit

                def vproj(j):
                    def emit():
                        ps = qkv_ps.tile([F, 512], F32, tag="qkv", name="qkvps")
                        out = ps[:, 0:P]          # [128 tok, 128 dim]
                        for c in range(ECH):
                            lhsT = xT[:, c, j * P * stride : (j + 1) * P * stride : stride]
                            nc.tensor.matmul(
                                out,
                                lhsT=lhsT,
                                rhs=wl_sb[:, l, 2, c, :],
                                start=(c == 0),
                                stop=(c == ECH - 1),
                            )
                        src = ps[:, 0:HD]
                        src.ap.insert(1, [HD, 2])
                        ch = CHOFF[l] + j
                        dstv = Vt[:, ch, 0:HD]
                        dstv.ap.insert(1, [HD + 2, 2])
                        nc.vector.tensor_copy(out=dstv, in_=src)

                    return emit

                work = []
                for part, dst in ((0, Q), (1, K)):
                    for n0 in range(0, sl, nt):
                        work.append(proj(part, n0, dst))
                for j in range(CH[l]):
                    work.append(vproj(j))
                return work

            def mk_filler(work, schedule):
                """Filler draining `work` with a per-call budget schedule."""
                state = {"i": 0}

                def filler():
                    k = (
                        schedule[state["i"]]
                        if state["i"] < len(schedule)
                        else (schedule[-1] if schedule else 0)
                    )
                    state["i"] += 1
                    for _ in range(min(k, len(work))):
                        work.pop(0)()

                return filler

            def drain(work):
                while work:
                    work.pop(0)()

            def attn_block(l, qb0, qbw, a_dst, a_off, filler=None, qstride=None):
                """Attention for q-block [qb0, qb0+qbw) of level l -> a_dst[:, a_off:].

                With qstride, the q-set is qbw//128 chunks of 128 tokens at
                stride qstride starting at qb0 (strided level-0 half-blocks).

                Emission interleaves score/exp units with the AV matmuls of
                the unit two steps back (matching the 2-buffer sc_ps
                rotation), so PE trails the exp stream by ~2us with no
                stall cascade.  The filler is called once before the first
                AV (for work that must precede AVs, e.g. V projections) and
                once after each AV group.
                """
                nch = CH[l]
                nqc = qbw // P
                fp8 = l == 0  # scores via fp8 DoubleRow (Q8 x (K_hi+K_lo))

                def qrhs(b, qc=None):
                    if qstride is None:
                        return Q[b : b + HD, LOFF[l] + qb0 : LOFF[l] + qb0 + qbw]
                    if qc is None:
                        r = Q[b : b + HD, LOFF[l] + qb0 : LOFF[l] + qb0 + P]
                        r.ap.insert(1, [qstride, nqc])
                        return r
                    # fp8 DoubleRow path: one contiguous 128-token q-chunk
                    q0 = qb0 + qc * qstride
                    return Q8[b : b + HD, :, q0 : q0 + P]

                avb = av_ps.tile([P, 8, P], F32, tag="avb")
                ats = {}
                seq = []
                for i0 in range(0, nch, 2):
                    for h in (0, 1):
                        seq.append((list(range(i0, min(i0 + 2, nch))), h))

                def emit_sc(pair, h):
                    b = h * HD
                    sp = sc_ps.tile([P, 2 * qbw], F32, tag="sc")
                    for j, kc in enumerate(pair):
                        if fp8:
                            for qc in range(nqc):
                                nc.tensor.matmul(
                                    sp[:, j * qbw + qc * P : j * qbw + (qc + 1) * P],
                                    lhsT=K8[b : b + HD, :, kc * P : (kc + 1) * P],
                                    rhs=qrhs(b, qc),
                                    start=True,
                                    stop=True,
                                    perf_mode=mybir.MatmulPerfMode.DoubleRow,
                                )
                        else:
                            nc.tensor.matmul(
                                sp[:, j * qbw : (j + 1) * qbw],
                                lhsT=K[b : b + HD, LOFF[l] + kc * P : LOFF[l] + (kc + 1) * P],
                                rhs=qrhs(b),
                                start=True,
                                stop=True,
                            )
                    at = at_pool.tile([P, 2 * qbw], BF16, tag="at")
                    nc.scalar.activation(
                        at[:, 0 : len(pair) * qbw],
                        sp[:, 0 : len(pair) * qbw],
                        mybir.ActivationFunctionType.Exp,
                    )
                    for j, kc in enumerate(pair):
                        ats[(kc, h)] = at[:, j * qbw : (j + 1) * qbw]

                def emit_av(pair, h):
                    for kc in pair:
                        for qc in range(nqc):
                            slot = qc * 2 + h
                            c0 = 0 if h == 0 else HD + 2
                            # one start per 2KB PSUM bank: the first write
                            # marks the whole bank pending-zero, the other
                            # slots' first writes self-zero.  stop on the
                            # chronologically last write to the bank.
                            nc.tensor.matmul(
                                avb[:, slot, 0 : HD + 1],
                                lhsT=ats[(kc, h)][:, qc * P : (qc + 1) * P],
                                rhs=Vt[:, CHOFF[l] + kc, c0 : c0 + HD + 1],
                                start=(kc == 0 and h == 0 and qc % 2 == 0),
                                stop=(
                                    kc == nch - 1
                                    and h == 1
                                    and (qc % 2 == 1 or qc == nqc - 1)
                                ),
                            )

                for i, (pair, h) in enumerate(seq):
                    emit_sc(pair, h)
                    if i == 1 and filler is not None:
                        filler()
                    if i >= 2:
                        emit_av(*seq[i - 2])
                        if filler is not None:
                            filler()
                for i in range(max(0, len(seq) - 2), len(seq)):
                    emit_av(*seq[i])
                    if filler is not None:
                        filler()

                # normalize (per-partition reciprocal of the denominator
                # column), pack both heads side by side, transpose back to
                # feature-major via the PE, copy into the destination buffer
                for qc in range(nqc):
                    sb = nrm_pool.tile([P, P], BF16, tag="sb")
                    for h in (0, 1):
                        slot = qc * 2 + h
                        r = nrm_pool.tile([P, 1], F32, tag=f"r{h}", name="rcp")
                        nc.vector.reciprocal(
                            r[:, 0:1], avb[:, slot, HD : HD + 1]
                        )
                        nc.vector.tensor_mul(
                            out=sb[:, h * HD : (h + 1) * HD],
                            in0=avb[:, slot, 0:HD],
                            in1=r[:, 0:1].to_broadcast((P, HD)),
                        )
                    tp = tr_ps.tile([P, P], BF16, tag="tr")
                    nc.tensor.transpose(tp[:], sb[:], ident[:])
                    nc.vector.tensor_copy(
                        out=a_dst[:, a_off + qc * P : a_off + (qc + 1) * P],
                        in_=tp[:],
                    )

            A123 = qk_pool.tile([P, CTOT], BF16, tag="A123")

            def pad_edges(l):
                sl = SL[l]
                co = CO[l]
                nc.vector.tensor_copy(
                    out=A123[:, co : co + PAD],
                    in_=A123[:, co + PAD : co + PAD + 1].to_broadcast((P, PAD)),
                )
                nc.vector.tensor_copy(
                    out=A123[:, co + PAD + sl : co + 2 * PAD + sl],
                    in_=A123[:, co + PAD + sl - 1 : co + PAD + sl].to_broadcast((P, PAD)),
                )

            def attn_level_whole(l, filler):
                sl = SL[l]
                co = CO[l]
                qbw = min(512, sl)
                for qb0 in range(0, sl, qbw):
                    attn_block(l, qb0, qbw, A123, co + PAD + qb0, filler=filler)
                pad_edges(l)

            def bounce_windows(l, dst_dram, woff):
                """One DMA materializing all 8 overlapping dest windows."""
                s0 = CO[l] + PAD - HALO[l]
                src = A123[:, s0 : s0 + WIN[l]]
                src.ap.insert(1, [BLK[l], NCORES])
                dst = dst_dram.ap().rearrange("d p w -> p d w")
                nc.sync.dma_start(dst[:, :, woff : woff + WIN[l]], src)

            def a2a(ins_t, outs_t):
                nc.gpsimd.collective_compute(
                    "AllToAll",
                    mybir.AluOpType.bypass,
                    replica_groups=rg,
                    ins=[ins_t[:]],
                    outs=[outs_t[:]],
                )

            def attn_level0_pass(filler, half, ag, gout):
                """One strided half-pass of level 0: the q-set is the
                `half`-th 128-token half of every dest core's 256-block."""
                for b in range(2):
                    A0 = a0_pool.tile([P, QB0], BF16, tag="A0")
                    attn_block(
                        0,
                        b * 4 * BLK[0] + half * HB,
                        QB0,
                        A0,
                        0,
                        filler=filler,
                        qstride=BLK[0],
                    )
                    # 4 dests x 128 cols per block
                    nc.sync.dma_start(
                        ag.ap()[b * 4 : (b + 1) * 4].rearrange("d p w -> p d w"),
                        A0[:].rearrange("p (d w) -> p d w", d=4),
                    )
                a2a(ag, gout)

            # ---------------- epilogue work units -------------------------
            # Z_l = (gathered A_l window) @ D_l at level-l resolution, per
            # 128-wide output feature tile; emitted as PE filler closures.
            def z_units(gtile, goff, l, w, fin, tail=False):
                """One closure per ft: matmul into psum, then fin(ft, ps).

                tail=True uses the (then idle) 2-buffer score PSUM pool so
                consecutive units pipeline instead of ping-ponging on the
                single qkv bank.
                """
                units = []
                for ft in range(FT):
                    def emit(ft=ft):
                        if tail:
                            ps = sc_ps.tile([P, 1024], F32, tag="sc", name="zps")
                        else:
                            ps = qkv_ps.tile([F, 512], F32, tag="qkv", name="qkvps")
                        for c in range(ECH):
                            nc.tensor.matmul(
                                ps[:, 0:w],
                                lhsT=wd_sb[:, l, c, ft],
                                rhs=gtile[:, c, goff : goff + w],
                                start=(c == 0),
                                stop=(c == ECH - 1),
                            )
                        fin(ft, ps[:, 0:w])
                    units.append(emit)
                return units

            def upsample(cur, ws, w, phase_a, tag):
                """2x linear-interp upsample [P, FT, ws] -> [P, FT, w] (DVE)."""
                p25 = acc_pool.tile([P, FT, ws], BF16, tag=f"p25{tag}")
                p75 = acc_pool.tile([P, FT, ws], BF16, tag=f"p75{tag}")
                nc.vector.tensor_scalar_mul(p25[:], cur[:], 0.25)
                nc.vector.tensor_scalar_mul(p75[:], cur[:], 0.75)
                up = acc_pool.tile([P, FT, w], BF16, tag=f"up{tag}")
                hw = (w + 1) // 2
                hw2 = w // 2
                if phase_a:
                    nc.vector.tensor_add(
                        up[:, :, 0::2], p25[:, :, 0:hw], p75[:, :, 1 : hw + 1]
                    )
                    nc.vector.tensor_add(
                        up[:, :, 1::2], p75[:, :, 1 : hw2 + 1], p25[:, :, 2 : hw2 + 2]
                    )
                else:
                    nc.vector.tensor_add(
                        up[:, :, 0::2], p75[:, :, 1 : hw + 1], p25[:, :, 2 : hw + 2]
                    )
                    nc.vector.tensor_add(
                        up[:, :, 1::2], p25[:, :, 1 : hw2 + 1], p75[:, :, 2 : hw2 + 2]
                    )
                return up

            # ---------------- schedule ------------------------------------
            # Attention order: 3, 2, 1, 0A, 0B.  Exchange order: a2a32,
            # a2a1, a2a0a, a2a0b -- each fires at data-ready into a free
            # slot on the Pool queue; only a2a0b is exposed at the tail.
            w3 = qkv_chunks(3)
            w2 = qkv_chunks(2)
            w1 = qkv_chunks(1)
            w0 = qkv_chunks(0)
            w1qk, w1v = w1[:4], w1[4:]
            w0qk, w0v = w0[:8], w0[8:]

            drain(w3)
            # level 3 attention; fill with level-2 QKV (6 units)
            attn_level_whole(3, mk_filler(w2, [2, 2, 2]))
            drain(w2)
            bounce_windows(3, agin32, 0)
            # level 2 attention; fill with level-1 Q/K
            attn_level_whole(2, mk_filler(w1qk, [0, 1, 1, 1, 1]))
            drain(w1qk)
            bounce_windows(2, agin32, WIN[3])
            a2a(agin32, g32)
            Gs32 = g_pool.tile([P, ECH, W32], BF16, tag="gs32")
            nc.sync.dma_start(Gs32[:], g32.ap().rearrange("b p t -> p b t"))

            # level 1 attention: V of L1 in the pre-AV slot (needed before
            # its AVs), then level-0 Q/K and level-0 V one unit per AV slot.
            wB = w1v + w0qk + w0v
            fB = mk_filler(wB, [8] + [1] * 30)
            attn_block(1, 0, 512, A123, CO[1] + PAD, filler=fB)
            attn_block(1, 512, 512, A123, CO[1] + PAD + 512, filler=fB)
            drain(wB)
            pad_edges(1)
            bounce_windows(1, agin1, 0)
            a2a(agin1, g1)
            Gs1 = g_pool.tile([P, ECH, WIN[1]], BF16, tag="gs1")
            nc.sync.dma_start(Gs1[:], g1.ap().rearrange("b p t -> p b t"))
            # folded weights for levels 1, 0 prefetch behind a2a1 on Pool
            for l in (1, 0):
                for c in range(ECH):
                    nc.gpsimd.dma_start(wd_sb[:, l, c], wd_r[:, l, c])

            # level 0 pass A; Z3 epilogue matmuls spread over the second
            # block's AV slots (Gs32 has landed by then).
            zt3 = g_pool.tile([P, FT, WIN[3]], BF16, tag="zt3")

            def fin3(ft, ps):
                nc.vector.tensor_copy(out=zt3[:, ft, :], in_=ps)

            z3u = z_units(Gs32, 0, 3, WIN[3], fin3)
            attn_level0_pass(mk_filler(z3u, [0] * 18 + [1] * 16), 0, agin0a, g0a)
            drain(z3u)
            wl_stack.close()
            acc_pool = pool("acc", 1)
            Gs0a = g_pool.tile([P, ECH, HB], BF16, tag="gs0a")
            nc.sync.dma_start(Gs0a[:], g0a.ap().rearrange("b p t -> p b t"))
            up_a = upsample(zt3, WIN[3], WIN[2], cfg["PHASE_A"][2], "a")

            # level 0 pass B with the Z2 then (up_b) then Z1 epilogue as
            # fillers; Gs1 lands early in this window.
            acc2 = acc_pool.tile([P, FT, WIN[2]], BF16, tag="acc2")
            acc1 = acc_pool.tile([P, FT, WIN[1]], BF16, tag="acc1")

            def fin2(ft, ps):
                nc.vector.tensor_tensor(
                    acc2[:, ft, :], ps, up_a[:, ft, :], mybir.AluOpType.add
                )

            def fin1(ft, ps):
                nc.vector.tensor_tensor(
                    acc1[:, ft, :], ps, up_b[:, ft, :], mybir.AluOpType.add
                )

            z2u = z_units(Gs32, WIN[3], 2, WIN[2], fin2)
            up_b = None

            def emit_up_b():
                nonlocal up_b
                up_b = upsample(acc2, WIN[2], WIN[1], cfg["PHASE_A"][1], "b")

            z1u = z_units(Gs1, 0, 1, WIN[1], fin1)
            wC = z2u + [emit_up_b] + z1u
            attn_level0_pass(mk_filler(wC, [0] + [1] * 40), 1, agin0b, g0b)
            drain(wC)

            # half-A epilogue + chain to upc while the last exchange flies
            o_a = acc_pool.tile([P, FT, HB], F32, tag="o_a")

            def fin0a(ft, ps):
                nc.vector.tensor_tensor(
                    o_a[:, ft, :],
                    ps,
                    beta_sb[:, ft : ft + 1].to_broadcast((P, HB)),
                    mybir.AluOpType.add,
                )

            drain(z_units(Gs0a, 0, 0, HB, fin0a, tail=True))
            upc = upsample(acc1, WIN[1], WIN[0], cfg["PHASE_A"][0], "c")

            # output half A: final add + streamed per-ft DMA.  The output
            # DMAs go on the Activation queue (its exps are done by now),
            # keeping SP free and the tail off the critical path.
            out_r = out_p.ap().rearrange("(c p) t -> p c t", p=P)
            for ft in range(FT):
                oa = acc_pool.tile([P, HB], F32, tag=f"oA{ft}", name="o_t")
                nc.vector.tensor_tensor(
                    oa[:], o_a[:, ft, :], upc[:, ft, 0:HB], mybir.AluOpType.add
                )
                nc.scalar.dma_start(out_r[:, ft, 0:HB], oa[:])

            # tail: half B (needs the last exchange); load split over two
            # DMA queues so it lands in ~0.8us.
            gs0b = g_pool.tile([P, ECH, HB], BF16, tag="gs0b")
            g0b_r = g0b.ap().rearrange("b p t -> p b t")
            nc.sync.dma_start(gs0b[:, 0:4], g0b_r[:, 0:4])
            nc.scalar.dma_start(gs0b[:, 4:8], g0b_r[:, 4:8])

            def fin0b(ft, ps):
                ob = acc_pool.tile([P, HB], F32, tag=f"oB{ft}", name="o_t")
                nc.vector.tensor_tensor(
                    ob[:],
                    ps,
                    beta_sb[:, ft : ft + 1].to_broadcast((P, HB)),
                    mybir.AluOpType.add,
                )
                nc.vector.tensor_tensor(
                    ob[:], ob[:], upc[:, ft, HB : 2 * HB], mybir.AluOpType.add
                )
                eng = nc.scalar if ft % 2 == 0 else nc.sync
                eng.dma_start(out_r[:, ft, HB : 2 * HB], ob[:])

            drain(z_units(gs0b, 0, 0, HB, fin0b, tail=True))

    nc.compile()
    return nc


# ---------------------------------------------------------------------------
# host-side input preparation / sharding
# ---------------------------------------------------------------------------

def make_in_maps(cfg, query, in_proj_w, in_proj_b, out_w, out_b, up_w, up_b):
    S, E, HD, F, ECH = cfg["S"], cfg["E"], cfg["HD"], cfg["F"], cfg["ECH"]
    FT = ECH
    f32 = np.float32
    f64 = np.float64

    query = np.asarray(query, f32)
    in_proj_w = np.asarray(in_proj_w, f32)
    in_proj_b = np.asarray(in_proj_b, f32)
    out_w = np.asarray(out_w, f32)
    out_b = np.asarray(out_b, f32)
    up_w = np.asarray(up_w, f32)
    up_b = np.asarray(up_b, f32)

    qT = np.ascontiguousarray(query[0].T.astype(BF16_NP))  # [E, S]

    # folded epilogue: D_l = W_out[l]^T @ up_w[l-1]^T @ ... @ up_w[0]^T
    # beta: beta_3 = b3; beta_l = beta_{l+1} @ up_w[l]^T + up_b[l] + b_l
    # V-bias fold: a V bias b_v shifts A_l by b_v (softmax weights sum to 1),
    # and the epilogue is linear, so it contributes b_v @ D_l to beta.
    D = []
    for l in range(LEVELS):
        M = out_w[l].T.astype(f64)
        for j in range(l - 1, -1, -1):
            M = M @ up_w[j].T.astype(f64)
        D.append(M.astype(f32))
    Dm = np.stack(D, axis=0)  # [L, E(in), E(out)] -- already W^T layout
    beta = out_b[3].astype(f64)
    for l in range(LEVELS - 2, -1, -1):
        beta = beta @ up_w[l].T.astype(f64) + up_b[l] + out_b[l]
    for l in range(LEVELS):
        bv = in_proj_b[l, 2 * E : 3 * E].astype(f64)
        beta = beta + bv @ D[l].astype(f64)
    beta = beta.astype(f32)

    # pack [L, e_in, e_out] -> [L, e_in%128, e_in//128, e_out//128, e_out%128]
    t = Dm.reshape(LEVELS, ECH, P, FT, P)          # [L, ec, ep, ft, fp]
    t = t.transpose(0, 2, 1, 3, 4)                 # [L, ep, ec, ft, fp]
    wd = np.ascontiguousarray(t.astype(BF16_NP))
    beta_pk = np.ascontiguousarray(beta.reshape(FT, P).T.astype(f32))  # [P, FT]

    scale = 1.0 / np.sqrt(HD).astype(f32)
    in_maps = []
    for c in range(NCORES):
        r0 = c * F
        sl_q = in_proj_w[:, r0 : r0 + F, :] * scale          # [L, F, E]
        sl_k = in_proj_w[:, E + r0 : E + r0 + F, :]
        sl_v = in_proj_w[:, 2 * E + r0 : 2 * E + r0 + F, :]
        w3 = np.stack([sl_q, sl_k, sl_v], axis=1)            # [L, 3, F, E]
        w3 = w3.transpose(0, 3, 1, 2)                        # [L, E(e), 3, F]
        w3 = w3.reshape(LEVELS, ECH, P, 3, F).transpose(0, 2, 3, 1, 4)
        w3 = np.ascontiguousarray(w3.astype(BF16_NP))        # [L, p, 3, ch, F]

        b_q = in_proj_b[:, r0 : r0 + F] * scale
        b_k = in_proj_b[:, E + r0 : E + r0 + F]
        b_v = np.zeros_like(b_k)  # folded into beta
        b3 = np.stack([b_q, b_k, b_v], axis=1)               # [L, 3, F]
        b3 = np.zeros((P, LEVELS, 3), f32) + b3.transpose(2, 0, 1)

        in_maps.append(
            {
                "qT": qT,
                "win": w3,
                "bin": np.ascontiguousarray(b3),
                "wd": wd,
                "beta": beta_pk,
            }
        )
    return in_maps


def assemble_output(cfg, results):
    S, E = cfg["S"], cfg["E"]
    blk = cfg["BLK"][0]
    out = np.empty((1, S, E), np.float32)
    for c in range(NCORES):
        out[0, c * blk : (c + 1) * blk, :] = results[c]["out"].T
    return out


_CACHE = {}


def _get_nc(cfg_key=(2048, 1024, 16)):
    if cfg_key not in _CACHE:
        cfg = _cfg(*cfg_key)
        _CACHE[cfg_key] = (cfg, build(cfg))
    return _CACHE[cfg_key]


def kernel(query, in_proj_w, in_proj_b, out_w, out_b, up_w, up_b):
    from concourse.bass_utils import run_bass_kernel_spmd

    cfg, nc = _get_nc()
    in_maps = make_in_maps(cfg, query, in_proj_w, in_proj_b, out_w, out_b, up_w, up_b)
    res = run_bass_kernel_spmd(nc, in_maps, core_ids=list(range(NCORES)))
    return assemble_output(cfg, res.results)
